# revision 23
# baseline (speedup 1.0000x reference)
"""Trainium2 Bass kernel for a single causal attention head.

Reference computation (per batch element b):
    Q = x_b @ wQ.T ; K = x_b @ wK.T ; V = x_b @ wV.T          [S, DK]
    P = softmax(causal_mask(Q @ K.T * d_model**-0.5))          [S, S]
    O = P @ V                                                  [S, DK]

Sharding: one batch element per NeuronCore (B == n_cores == 8).
Weights are replicated. No collectives needed.

Per-core device layout (host pre-transposes/casts for PE-friendly fp16):
    xt    [D, S]   fp16  x_b transposed (contraction dim D on partitions)
    wqk   [D, 128] fp16  [wQ.T | wK.T]  -> packed projection, M=128
    wv    [D, 64]  fp16  wV.T
Output:
    o     [NQ, 65, QC] fp32  unnormalized U^T rows 0:64 + softmax
                             denominators in row 64; the host divides and
                             transposes during the gather/unshard step
                             (flash-attention partial-merge convention).

Device pipeline per 512-wide q-chunk c (fp16 matmuls, fp32 PSUM):
  - DMA xt chunk; project Q^T,K^T packed (M=128); project V^T with
    col-tiled half-chunks into one PSUM tile.
  - Duplicate Q^T/K^T across both partition halves (SBUF->SBUF DMA) so
    score matmuls for t-tiles j,j+1 run row-packed (rows 0-63 / 64-127
    of the PE array concurrently), writing one [128,1024] PSUM pair.
  - exp: the ScalarE ACTIVATE is the kernel bottleneck, so the exp work
    is split across two engines: even steady pairs use the ScalarE exp
    ACTIVATE -> fp8; odd steady pairs compute exp on the Vector engine
    with the Schraudolph exp2 bit trick (int8(z*8/ln2 + bias) bitcast
    to fp8e4m3 ~= 2^z with ~3% multiplicative ripple that washes out in
    the softmax average).
  - Diagonal pairs: ScalarE exp -> fp8, causal-masked in place by
    GpSimd affine_selects (keeps the Vector engine free).
  - PV: accumulate [V_j | 1].T @ P~_j into PSUM (U^T rows 0-63,
    softmax denominators in row 64) with fp8 DoubleRow matmuls.
"""

import numpy as np
import ml_dtypes

B, S, D, DK = 8, 4096, 1024, 64
P = 128
QC = 512          # q-chunk width (matmul moving dim)
NQ = S // QC      # 8 q-chunks
ND = D // P       # 8 contraction chunks
NT = S // P       # 32 t-tiles
SCALE = float(D) ** -0.5   # 1/32
VW = 66           # per-t-tile stride in v_sb (64 V cols + 1 ones + pad)

# Schraudolph exp2-bit-trick constants for the DVE exp path:
# i8 = trunc(score * SCHRA_A + SCHRA_B); bitcast(i8) as fp8e4m3 ~ exp(score/32)
SCHRA_A = SCALE * 8.0 * 1.4426950408889634
SCHRA_B = 55.95

# optimization knobs (validated on HW; flip off if a variant fails)
ROWPACK_SCORES = True   # row-packed score matmul pairs
SCHRAB = True           # split each pair's exp: ScalarE half + DVE half
GMASK = True            # diagonal-pair causal masks on GpSimd
THETA = 512             # exp split point: ScalarE gets [0:THETA) of 1024

_CACHED = {}


def _build_nc():
    import concourse.mybir as mybir
    import concourse.tile as tile
    from concourse import bacc
    from concourse.masks import make_identity
    from contextlib import ExitStack

    f32 = mybir.dt.float32
    f16 = mybir.dt.float16
    f8 = mybir.dt.float8e4
    i8 = mybir.dt.int8
    DR = mybir.MatmulPerfMode.DoubleRow
    Exp = mybir.ActivationFunctionType.Exp
    add_op = mybir.AluOpType.add
    mult = mybir.AluOpType.mult
    is_gt = mybir.AluOpType.is_gt

    nc = bacc.Bacc()
    # host pre-layouts: per-partition contiguous so DMAs need no gather.
    # ww packs [wqk | wv] along the last dim -> one DMA issue.
    WW = P + DK
    xt_h = nc.declare_dram_parameter("xt", [P, NQ, ND, QC], f16, isOutput=False)
    ww_h = nc.declare_dram_parameter("ww", [P, ND, WW], f16, isOutput=False)
    ch_h = nc.declare_dram_parameter("chain", [P, P], f32, isOutput=False)
    o_h = nc.declare_dram_parameter("o", [NQ, 65, QC], f32, isOutput=True)
    cho_h = nc.declare_dram_parameter("chain_o", [P, P], f32, isOutput=True)

    with tile.TileContext(nc) as tc, ExitStack() as ctx:
        const = ctx.enter_context(tc.tile_pool(name="const", bufs=1))
        xt_pool = ctx.enter_context(tc.tile_pool(name="xtp", bufs=2))
        pers = ctx.enter_context(tc.tile_pool(name="pers", bufs=1))
        pt_pool = ctx.enter_context(tc.tile_pool(name="ptp", bufs=3))
        pt8_pool = ctx.enter_context(tc.tile_pool(name="ptp8", bufs=6))
        stage = ctx.enter_context(tc.tile_pool(name="stage", bufs=2))
        # PSUM budget (8 banks): pair pool 3x2 + po 1 + sm 1 = 8.  Three
        # score buffers let the PE run up to three pairs ahead of the exp
        # engines, hiding per-matmul pipeline latency and exp jitter.
        ps_pair = ctx.enter_context(tc.tile_pool(name="ps_pair", bufs=3, space="PSUM"))
        ps_op = ctx.enter_context(tc.tile_pool(name="ps_op", bufs=1, space="PSUM"))
        ps_sm = ctx.enter_context(tc.tile_pool(name="ps_sm", bufs=1, space="PSUM"))

        # ---- weights first (small, gate the first projection), then the
        # first x chunk in quarters so the projection starts on the first
        # quarter while the rest streams in ----
        ww_sb = const.tile([P, ND, WW], f16)
        nc.sync.dma_start(out=ww_sb, in_=ww_h[:])
        wqk_sb = ww_sb[:, :, 0:P]
        wv_sb = ww_sb[:, :, P:]
        xtc0 = xt_pool.tile([P, ND, QC], f16, name="xtc", tag="xtc")
        for h in range(4):
            nc.sync.dma_start(
                out=xtc0[:, 2 * h:2 * (h + 1), :],
                in_=xt_h[:, 0, 2 * h:2 * (h + 1), :],
            )
        # ---- PE warm-up: dummy matmuls during the initial DMA wait so the
        # HAM clock gate is already at full rate (2.4 GHz) when the first
        # projection runs.  Depends only on one vector memset. ----
        warm_sb = const.tile([P, QC], f16, name="warm_sb")
        nc.vector.memset(warm_sb, 0.0)
        ps_warm = ps_sm.tile([P, QC], f32, name="ps_warm", tag="sm")
        for _ in range(3):
            nc.tensor.matmul(ps_warm, lhsT=warm_sb[:, 0:P], rhs=warm_sb,
                             start=True, stop=True)
        # causal masks: fp16 multiplicative (post-exp) for the fp16-path
        # pairs, plus an fp32 additive bias (pre-exp) used only on the final
        # trimmed pair where it shortens the drained tail chain
        dm_sb = const.tile([P, 2, 2 * QC], f16)
        nc.gpsimd.memset(dm_sb, 0.0)
        for g in range(2):
            for h in range(2):
                nc.gpsimd.affine_select(
                    out=dm_sb[:, g, h * QC:(h + 1) * QC],
                    in_=dm_sb[:, g, h * QC:(h + 1) * QC],
                    compare_op=is_gt,
                    fill=1.0,
                    base=P * (2 * g + h),
                    pattern=[[-1, QC]],
                    channel_multiplier=1,
                )
        mb_sb = const.tile([P, 2 * QC], f32)
        nc.gpsimd.memset(mb_sb, 0.0)
        for h in range(2):
            # complement of the live predicate (expr<=0) via a negated
            # iota, since only is_gt has codegen support
            nc.gpsimd.affine_select(
                out=mb_sb[:, h * QC:(h + 1) * QC],
                in_=mb_sb[:, h * QC:(h + 1) * QC],
                compare_op=is_gt,
                fill=-1.0e5,
                base=1 - P * (2 + h),
                pattern=[[1, QC]],
                channel_multiplier=-1,
            )
        # contiguous copy of the final trimmed pair's live-region mask so
        # the tail masking is a single tensor_tensor over [QC/2, QC+QC/4)
        mbt_sb = const.tile([P, 3 * QC // 4], f32)
        nc.vector.tensor_copy(mbt_sb[:, 0:QC // 2], mb_sb[:, QC // 2:QC])
        nc.vector.tensor_copy(
            mbt_sb[:, QC // 2:], mb_sb[:, 2 * QC - QC // 4:]
        )
        if not GMASK:
            dm8_sb = const.tile([P, 2 * QC], f8)
            nc.vector.tensor_copy(dm8_sb, dm_sb[:, 0, :])
        # tiny pass-through used to chain executions when benchmarking
        cht = const.tile([P, P], f32, name="cht")
        nc.scalar.dma_start(out=cht, in_=ch_h[:])
        nc.scalar.dma_start(out=cho_h[:], in_=cht)

        # ---- persistent activations ----
        qk_sb = pers.tile([P, S], f16)    # rows 0:64 Q^T, rows 64:128 K^T
        kt2_sb = pers.tile([64, S], f16)  # K^T relocated to partitions 0-63
        if ROWPACK_SCORES:
            qt2_sb = pers.tile([P, S], f16)  # rows 64:128 = Q^T duplicate
        v_sb = pers.tile([P, NT, VW], f16)  # V natural tiles + ones column
        nc.vector.memset(v_sb[:, :, 64:65], 1.0)
        # fp8 copy of V, DoubleRow-interleaved by tile pair (+ ones col):
        # steady-state (non-diagonal) PV runs as one fp8 DoubleRow matmul
        # per pair, halving its PE stream time.  Softmax averaging over
        # >=512 keys keeps the fp8 V/P noise ~0.2% on those pairs; the
        # few-key diagonal pairs stay on the fp16 path.
        v8_sb = pers.tile([P, NT // 2, 2, 80], f8)
        nc.vector.memset(v8_sb[:, :, :, 64:65], 1.0)

        xtc_tiles = {}
        po_tiles = {}

        def emit_xtc_dma(c):
            if c >= NQ:
                return
            t = xt_pool.tile([P, ND, QC], f16, name="xtc", tag="xtc")
            nc.sync.dma_start(out=t, in_=xt_h[:, c, :, :])
            xtc_tiles[c] = t

        def qk_store(c, ps_qk):
            """PSUM->SBUF move of packed Q^T/K^T on ScalarE (the DVE queue
            carries the latency-critical Schraudolph exp halves)."""
            cs = slice(c * QC, (c + 1) * QC)
            nc.scalar.copy(qk_sb[:, cs], ps_qk)
            # relocations (partition shifts need a DMA, not a DVE op); on
            # the sync ring so the gpsimd queue stays free for the
            # latency-critical diagonal masks
            nc.sync.dma_start(out=kt2_sb[:, cs], in_=qk_sb[64:128, cs])
            if ROWPACK_SCORES:
                nc.sync.dma_start(out=qt2_sb[64:128, cs], in_=qk_sb[0:64, cs])

        def proj0_gen():
            """Chunk-0 projection (fp16)."""
            xtc = xtc_tiles.pop(0)
            ps_qk = ps_sm.tile([P, QC], f32, name="ps_qk", tag="sm")
            for dc in range(ND):
                nc.tensor.matmul(
                    ps_qk, lhsT=wqk_sb[:, dc, :], rhs=xtc[:, dc, :],
                    start=(dc == 0), stop=(dc == ND - 1),
                )
                if dc % 2 == 1:
                    yield
            qk_store(0, ps_qk)
            yield
            # V projection: two col-tiled halves into one PSUM tile
            ps_v = ps_sm.tile([P, QC // 2], f32, name="ps_v", tag="sm")
            for dc in range(ND):
                st, sp = (dc == 0), (dc == ND - 1)
                nc.tensor.matmul(
                    ps_v[0:64, :], lhsT=wv_sb[:, dc, :],
                    rhs=xtc[:, dc, 0:QC // 2], start=st, stop=sp,
                )
                nc.tensor.matmul(
                    ps_v[64:128, :], lhsT=wv_sb[:, dc, :],
                    rhs=xtc[:, dc, QC // 2:], start=st, stop=sp,
                    tile_position=(0, 64),
                )
                if dc % 4 == 3:
                    yield
            vt_sb = stage.tile([P, QC // 2], f16, name="vt_sb", tag="vt")
            nc.vector.tensor_copy(vt_sb, ps_v)
            yield
            yield from v_epilog(0, vt_sb)
            emit_xtc_dma(2)

        def v_epilog(c, vt_sb):
            """Shared V-projection tail: transposes -> v_sb/v8_sb tiles.
            The [128,128] transposes run on the DMA xbar engine instead of
            the PE, freeing PE stream time; the copies then read SBUF
            (faster DVE perf modes than PSUM sources)."""
            # transpose [128,128] once per half: rows 0:64 of the result
            # are t-tile 4c+k, rows 64:128 are t-tile 4c+2+k
            for k in range(2):
                tv_sb = stage.tile([P, P], f16, name="tv_sb", tag="tv")
                nc.sync.dma_start_transpose(tv_sb, vt_sb[:, k * P:(k + 1) * P])
                src2 = tv_sb[:, :].rearrange("p (a b) -> p a b", a=2)
                # merged strided copies: one op covers both t-tiles
                nc.vector.tensor_copy(
                    v_sb[:, 4 * c + k:4 * c + k + 3:2, 0:64], src2
                )
                nc.vector.tensor_copy(
                    v8_sb[:, 2 * c:2 * c + 2, k, 0:64], src2
                )
                yield

        def qkproj_gen(c):
            """Q,K projection of chunk c>=1 (packed, M=128)."""
            xtc = xtc_tiles[c]
            ps_qk = ps_sm.tile([P, QC], f32, name="ps_qk", tag="sm")
            for dc in range(ND):
                nc.tensor.matmul(
                    ps_qk, lhsT=wqk_sb[:, dc, :], rhs=xtc[:, dc, :],
                    start=(dc == 0), stop=(dc == ND - 1),
                )
                if dc % 2 == 1:
                    yield
            qk_store(c, ps_qk)
            yield

        def vproj_gen(c):
            """V projection of chunk c>=1: two col-tiled halves."""
            xtc = xtc_tiles.pop(c)
            ps_v = ps_sm.tile([P, QC // 2], f32, name="ps_v", tag="sm")
            for dc in range(ND):
                st, sp = (dc == 0), (dc == ND - 1)
                nc.tensor.matmul(
                    ps_v[0:64, :], lhsT=wv_sb[:, dc, :],
                    rhs=xtc[:, dc, 0:QC // 2], start=st, stop=sp,
                )
                nc.tensor.matmul(
                    ps_v[64:128, :], lhsT=wv_sb[:, dc, :],
                    rhs=xtc[:, dc, QC // 2:], start=st, stop=sp,
                    tile_position=(0, 64),
                )
                if dc % 4 == 3:
                    yield
            vt_sb = stage.tile([P, QC // 2], f16, name="vt_sb", tag="vt")
            nc.vector.tensor_copy(vt_sb, ps_v)
            yield
            yield from v_epilog(c, vt_sb)
            emit_xtc_dma(c + 2)

        def emit_pair(c, jp, po):
            """Emit the score matmuls + exp (+ masks) for pair jp of chunk
            c.  Returns a closure that emits the PV matmul(s); the caller
            defers it by two pairs so the PE queue always has ready score
            work ahead of a PV that stalls on its exp -- an in-order queue
            would otherwise expose the PE pipeline-fill latency on every
            exp wait."""
            cs = slice(c * QC, (c + 1) * QC)
            njs = 4 * (c + 1)
            j0, j1 = 2 * jp, 2 * jp + 1
            jj = j0 - 4 * c
            trimmed = jj == 2  # second diagonal pair: >62% masked
            # pair 0 avoids the row-packed path so a fresh chunk's first
            # scores don't wait on the qt2 relocation DMA
            packed = ROWPACK_SCORES and jp > 0

            def ktlo(j):
                return kt2_sb[:, j * P:(j + 1) * P]

            def qthi(lo):
                return qt2_sb[64:128, c * QC + lo:(c + 1) * QC]

            ps_s = ps_pair.tile([P, 2 * QC], f32, name="ps_s", tag="pair")
            if trimmed or (jj == 0 and c == 0):
                pt = pt_pool.tile([P, 2 * QC], f16, name="pt", tag="pt")
            if trimmed:
                # jj=2 half: only q in [256,512) is live; jj=3 half: only q in
                # [384,512), remapped to columns [512,640) so one ACT covers a
                # contiguous [256,640) region.
                nc.tensor.matmul(
                    ps_s[:, QC // 2:QC],
                    lhsT=ktlo(j0),
                    rhs=qk_sb[0:64, c * QC + QC // 2:(c + 1) * QC],
                    start=True, stop=True,
                )
                nc.tensor.matmul(
                    ps_s[:, QC:QC + QC // 4],
                    lhsT=qk_sb[64:128, j1 * P:(j1 + 1) * P],
                    rhs=qthi(3 * QC // 4),
                    start=True, stop=True, tile_position=(64, 0),
                )
                nc.gpsimd.memset(pt[:, 0:QC // 2], 0.0)
                if c == NQ - 1:
                    # final pair: mask pre-exp (drained pipeline; shortens
                    # the serial ACT->mask->PV tail chain)
                    nc.vector.tensor_tensor(
                        ps_s[:, QC // 2:QC + QC // 4],
                        ps_s[:, QC // 2:QC + QC // 4], mbt_sb, op=add_op,
                    )
                    nc.scalar.activation(
                        pt[:, QC // 2:QC + QC // 4],
                        ps_s[:, QC // 2:QC + QC // 4], Exp, scale=SCALE,
                    )
                else:
                    nc.scalar.activation(
                        pt[:, QC // 2:QC + QC // 4],
                        ps_s[:, QC // 2:QC + QC // 4], Exp, scale=SCALE,
                    )
                    # causal trimming on GpSimd (both slices reduce to the
                    # same local predicate: live where col >= partition)
                    for off, w in ((QC // 2, QC // 2), (QC, QC // 4)):
                        nc.gpsimd.affine_select(
                            out=pt[:, off:off + w], in_=pt[:, off:off + w],
                            compare_op=is_gt, fill=0.0,
                            base=1, pattern=[[1, w]],
                            channel_multiplier=-1,
                        )
                def pv_trimmed():
                    nc.tensor.matmul(
                        po[:, 3 * QC // 4:], lhsT=v_sb[:, j1, 0:65],
                        rhs=pt[:, QC:QC + QC // 4], start=False, stop=False,
                    )
                    nc.tensor.matmul(
                        po, lhsT=v_sb[:, j0, 0:65], rhs=pt[:, 0:QC],
                        start=False, stop=(j1 == njs - 1),
                    )
                return pv_trimmed
            nc.tensor.matmul(
                ps_s[:, 0:QC],
                lhsT=ktlo(j0), rhs=qk_sb[0:64, cs],
                start=True, stop=True,
            )
            if packed:
                nc.tensor.matmul(
                    ps_s[:, QC:],
                    lhsT=qk_sb[64:128, j1 * P:(j1 + 1) * P],
                    rhs=qthi(0),
                    start=True, stop=True, tile_position=(64, 0),
                )
            else:
                nc.tensor.matmul(
                    ps_s[:, QC:],
                    lhsT=ktlo(j1), rhs=qk_sb[0:64, cs],
                    start=True, stop=True,
                )
            if jj == 0 and c == 0:  # chunk-0 diagonal pair: fp16 + mask
                nc.scalar.activation(pt, ps_s, Exp, scale=SCALE)
                nc.vector.tensor_tensor(pt, pt, dm_sb[:, 0, :], op=mult)

                def pv_c0():
                    nc.tensor.matmul(
                        po, lhsT=v_sb[:, j0, 0:65], rhs=pt[:, 0:QC],
                        start=(j0 == 0), stop=False,
                    )
                    nc.tensor.matmul(
                        po, lhsT=v_sb[:, j1, 0:65], rhs=pt[:, QC:],
                        start=False, stop=(j1 == njs - 1),
                    )
                return pv_c0
            # fp8 exp tile, flat [P, 2QC]; the PV matmul reads it through a
            # DoubleRow-interleaved [P, 2, QC] view
            pt8 = pt8_pool.tile([P, 2 * QC], f8, name="pt8", tag="pt8")
            diag = jj == 0  # c >= 1 here (c == 0 handled above)
            if SCHRAB:
                # split exp: ScalarE ACTIVATE on the first THETA columns,
                # DVE Schraudolph exp2 bit trick on the rest.  Both run
                # concurrently, halving the per-pair exp latency and
                # splitting the elementwise load across the two engines.
                th = QC if diag else THETA
                nc.scalar.activation(
                    pt8[:, 0:th], ps_s[:, 0:th], Exp, scale=SCALE
                )
                nc.vector.tensor_scalar(
                    pt8[:, th:].bitcast(i8), ps_s[:, th:],
                    SCHRA_A, SCHRA_B, op0=mult, op1=add_op,
                )
            else:
                nc.scalar.activation(pt8, ps_s, Exp, scale=SCALE)
            if diag:
                if GMASK:
                    # in-place causal zeroing on GpSimd: keeps where
                    # query >= key, fills 0 above the diagonal
                    for ko in range(2):
                        nc.gpsimd.affine_select(
                            out=pt8[:, ko * QC:(ko + 1) * QC],
                            in_=pt8[:, ko * QC:(ko + 1) * QC],
                            compare_op=is_gt, fill=0.0,
                            base=1 - P * ko,
                            pattern=[[1, QC]],
                            channel_multiplier=-1,
                        )
                else:
                    nc.vector.tensor_tensor(pt8, pt8, dm8_sb, op=mult)

            def pv_dr():
                nc.tensor.matmul(
                    po, lhsT=v8_sb[:, jp, :, 0:65],
                    rhs=pt8[:, :].rearrange("p (a b) -> p a b", a=2),
                    start=(j0 == 0), stop=(j1 == njs - 1),
                    perf_mode=DR,
                )
            return pv_dr

        def epi_gen(c):
            """Store U^T + denominators for chunk c; the host normalizes."""
            last = c == NQ - 1
            po = po_tiles.pop(c)
            osb = stage.tile([65, QC], f32, name="osb", tag="osb")
            nc.scalar.copy(osb, po)
            yield
            eng = nc.scalar if last else nc.sync
            eng.dma_start(out=o_h[c], in_=osb)
            yield

        # Software pipeline: a global queue of deferrable PE work
        # (projections of later chunks, epilogues of finished chunks) is
        # drained in small bursts between attention pairs, so the PE fills
        # its exp-wait slack and never idles across chunk boundaries.
        proj_pending = []   # [(due, generator)] sorted by deadline
        epi_pending = []    # generators (no deadline)

        def pull_one(max_due=None):
            while proj_pending:
                due, g = proj_pending[0]
                if max_due is not None and due > max_due:
                    break
                try:
                    next(g)
                    return
                except StopIteration:
                    proj_pending.pop(0)
            while epi_pending:
                try:
                    next(epi_pending[0])
                    return
                except StopIteration:
                    epi_pending.pop(0)

        def ensure(due):
            while proj_pending and proj_pending[0][0] <= due:
                _, g = proj_pending[0]
                for _ in g:
                    pass
                proj_pending.pop(0)

        xtc_tiles[0] = xtc0
        emit_xtc_dma(1)
        for _ in proj0_gen():
            pass
        # deadline-ordered deferred PE work: qkproj(c) is due at chunk-c
        # start, vproj(c) only at chunk c's first diagonal pair (pair 2c),
        # which spreads projection bursts across the ACT-paced pair loop
        for c in range(1, NQ):
            proj_pending.append(((c, 0), qkproj_gen(c)))
            proj_pending.append(((c, 1), vproj_gen(c)))
        proj_pending.sort(key=lambda t: t[0])
        for c in range(NQ):
            ensure((c, 0))
            po = ps_op.tile([65, QC], f32, name="po", tag="po")
            po_tiles[c] = po
            pv_queue = []
            for jp in range(2 * (c + 1)):
                if jp == 2 * c:
                    ensure((c, 1))
                pv_queue.append(emit_pair(c, jp, po))
                if len(pv_queue) > 2:
                    pv_queue.pop(0)()
                # front-load upcoming projections into the pair slack so
                # they (and the relocation DMAs) beat their deadlines
                pulls = 3 if jp < 4 else 1
                for _ in range(pulls):
                    pull_one(max_due=(c + 1, 0))
            while pv_queue:
                pv_queue.pop(0)()
            epi_pending.append(epi_gen(c))
        for _, g in proj_pending:
            for _ in g:
                pass
        proj_pending.clear()
        for g in epi_pending:
            for _ in g:
                pass
    nc.finalize()
    return nc


def _dev_w(w):
    # [D, m] -> [P, ND, m] with d = dc*P + p
    w = np.ascontiguousarray(w).astype(np.float16)
    return np.ascontiguousarray(w.reshape(ND, P, -1).transpose(1, 0, 2))


def _host_inputs(x, wQ, wK, wV):
    x = np.asarray(x, dtype=np.float32)
    wqk = _dev_w(np.concatenate([np.asarray(wQ).T, np.asarray(wK).T], axis=1))
    wv = _dev_w(np.asarray(wV).T)
    ww = np.ascontiguousarray(np.concatenate([wqk, wv], axis=2))
    chain = np.zeros((P, P), np.float32)
    in_maps = []
    for b in range(B):
        # x_b.T [D, S] -> [P, NQ, ND, QC] with d = dc*P + p, s = c*QC + sc
        xt = np.ascontiguousarray(
            x[b].T.astype(np.float16)
            .reshape(ND, P, NQ, QC)
            .transpose(1, 2, 0, 3)
        )
        in_maps.append({"xt": xt, "ww": ww, "chain": chain})
    return in_maps


def kernel(x, wQ, wK, wV):
    from concourse.bass_utils import run_bass_kernel_spmd

    if "nc" not in _CACHED:
        _CACHED["nc"] = _build_nc()
    nc = _CACHED["nc"]
    in_maps = _host_inputs(x, wQ, wK, wV)
    res = run_bass_kernel_spmd(nc, in_maps, core_ids=list(range(B)))
    out = np.empty((B, S, DK), np.float32)
    for b in range(B):
        u = res.results[b]["o"]          # [NQ, 65, QC]
        o = u[:, :64, :] / u[:, 64:65, :]
        out[b] = o.transpose(0, 2, 1).reshape(S, DK)
    return out


# revision 28
# speedup vs baseline: 1.0919x; 1.0919x over previous
"""Trainium2 Bass kernel for a single causal attention head.

Reference computation (per batch element b):
    Q = x_b @ wQ.T ; K = x_b @ wK.T ; V = x_b @ wV.T          [S, DK]
    P = softmax(causal_mask(Q @ K.T * d_model**-0.5))          [S, S]
    O = P @ V                                                  [S, DK]

Sharding: one batch element per NeuronCore (B == n_cores == 8).
Weights are replicated. No collectives needed.

Per-core device layout (host pre-transposes/casts for PE-friendly fp16):
    xt    [D, S]   fp16  x_b transposed (contraction dim D on partitions)
    wqk   [D, 128] fp16  [wQ.T | wK.T]  -> packed projection, M=128
    wv    [D, 64]  fp16  wV.T
Output:
    o     [NQ, 65, QC] fp32  unnormalized U^T rows 0:64 + softmax
                             denominators in row 64; the host divides and
                             transposes during the gather/unshard step
                             (flash-attention partial-merge convention).

Device pipeline per 512-wide q-chunk c (fp16 matmuls, fp32 PSUM):
  - DMA xt chunk; project Q^T,K^T packed (M=128); project V^T with
    col-tiled half-chunks into one PSUM tile.
  - Duplicate Q^T/K^T across both partition halves (SBUF->SBUF DMA) so
    score matmuls for t-tiles j,j+1 run row-packed (rows 0-63 / 64-127
    of the PE array concurrently), writing one [128,1024] PSUM pair.
  - exp: the ScalarE ACTIVATE is the kernel bottleneck, so the exp work
    is split across two engines: even steady pairs use the ScalarE exp
    ACTIVATE -> fp8; odd steady pairs compute exp on the Vector engine
    with the Schraudolph exp2 bit trick (int8(z*8/ln2 + bias) bitcast
    to fp8e4m3 ~= 2^z with ~3% multiplicative ripple that washes out in
    the softmax average).
  - Diagonal pairs: ScalarE exp -> fp8, causal-masked in place by
    GpSimd affine_selects (keeps the Vector engine free).
  - PV: accumulate [V_j | 1].T @ P~_j into PSUM (U^T rows 0-63,
    softmax denominators in row 64) with fp8 DoubleRow matmuls.
"""

import numpy as np
import ml_dtypes

B, S, D, DK = 8, 4096, 1024, 64
P = 128
QC = 512          # q-chunk width (matmul moving dim)
NQ = S // QC      # 8 q-chunks
ND = D // P       # 8 contraction chunks
NT = S // P       # 32 t-tiles
SCALE = float(D) ** -0.5   # 1/32
VW = 66           # per-t-tile stride in v_sb (64 V cols + 1 ones + pad)

# Schraudolph exp2-bit-trick constants for the DVE exp path:
# i8 = trunc(score * SCHRA_A + SCHRA_B); bitcast(i8) as fp8e4m3 ~ exp(score/32)
SCHRA_A = SCALE * 8.0 * 1.4426950408889634
SCHRA_B = 55.95

# optimization knobs (validated on HW; flip off if a variant fails)
ROWPACK_SCORES = True   # row-packed score matmul pairs
SCHRAB = True           # split each pair's exp: ScalarE half + DVE half
GMASK = True            # diagonal-pair causal masks on GpSimd
THETA = 512             # exp split point: ScalarE gets [0:THETA) of 1024

_CACHED = {}


def _build_nc():
    import concourse.mybir as mybir
    import concourse.tile as tile
    from concourse import bacc
    from concourse.masks import make_identity
    from contextlib import ExitStack

    f32 = mybir.dt.float32
    f16 = mybir.dt.float16
    f8 = mybir.dt.float8e4
    i8 = mybir.dt.int8
    DR = mybir.MatmulPerfMode.DoubleRow
    Exp = mybir.ActivationFunctionType.Exp
    add_op = mybir.AluOpType.add
    mult = mybir.AluOpType.mult
    is_gt = mybir.AluOpType.is_gt

    nc = bacc.Bacc()
    # host pre-layouts: per-partition contiguous so DMAs need no gather.
    # ww packs [wqk | wv] along the last dim -> one DMA issue.
    WW = P + DK
    xt_h = nc.declare_dram_parameter("xt", [P, NQ, ND, QC], f16, isOutput=False)
    ww_h = nc.declare_dram_parameter("ww", [P, ND, WW], f16, isOutput=False)
    ch_h = nc.declare_dram_parameter("chain", [P, P], f32, isOutput=False)
    o_h = nc.declare_dram_parameter("o", [NQ, 65, QC], f32, isOutput=True)
    cho_h = nc.declare_dram_parameter("chain_o", [P, P], f32, isOutput=True)

    with tile.TileContext(nc) as tc, ExitStack() as ctx:
        const = ctx.enter_context(tc.tile_pool(name="const", bufs=1))
        xt_pool = ctx.enter_context(tc.tile_pool(name="xtp", bufs=2))
        pers = ctx.enter_context(tc.tile_pool(name="pers", bufs=1))
        pt_pool = ctx.enter_context(tc.tile_pool(name="ptp", bufs=3))
        pt8_pool = ctx.enter_context(tc.tile_pool(name="ptp8", bufs=6))
        stage = ctx.enter_context(tc.tile_pool(name="stage", bufs=2))
        # PSUM budget (8 banks): pair pool 3x2 + po 1 + sm 1 = 8.  Three
        # score buffers let the PE run up to three pairs ahead of the exp
        # engines, hiding per-matmul pipeline latency and exp jitter.
        ps_pair = ctx.enter_context(tc.tile_pool(name="ps_pair", bufs=3, space="PSUM"))
        ps_op = ctx.enter_context(tc.tile_pool(name="ps_op", bufs=1, space="PSUM"))
        ps_sm = ctx.enter_context(tc.tile_pool(name="ps_sm", bufs=1, space="PSUM"))

        # ---- weights first (small, gate the first projection), then the
        # first x chunk in quarters so the projection starts on the first
        # quarter while the rest streams in ----
        ww_sb = const.tile([P, ND, WW], f16)
        nc.sync.dma_start(out=ww_sb, in_=ww_h[:])
        wqk_sb = ww_sb[:, :, 0:P]
        wv_sb = ww_sb[:, :, P:]
        xtc0 = xt_pool.tile([P, ND, QC], f16, name="xtc", tag="xtc")
        for h in range(4):
            nc.sync.dma_start(
                out=xtc0[:, 2 * h:2 * (h + 1), :],
                in_=xt_h[:, 0, 2 * h:2 * (h + 1), :],
            )
        # ---- PE warm-up: dummy matmuls during the initial DMA wait so the
        # HAM clock gate is already at full rate (2.4 GHz) when the first
        # projection runs.  Depends only on one vector memset. ----
        warm_sb = const.tile([P, QC], f16, name="warm_sb")
        nc.vector.memset(warm_sb, 0.0)
        # warm in the (still unused) pair pool so the warmup drain never
        # blocks the first projection's ps_sm bank
        ps_warm = ps_pair.tile([P, 2 * QC], f32, name="ps_warm", tag="pair")
        for _ in range(3):
            nc.tensor.matmul(ps_warm[:, 0:QC], lhsT=warm_sb[:, 0:P],
                             rhs=warm_sb, start=True, stop=True)
        # identity before the causal masks: ident16 gates the first V
        # transpose (PE critical path)
        ident16 = const.tile([P, P], f16)
        make_identity(nc, ident16)
        # causal masks: fp16 multiplicative (post-exp) for the fp16-path
        # pairs, plus an fp32 additive bias (pre-exp) used only on the final
        # trimmed pair where it shortens the drained tail chain
        dm_sb = const.tile([P, 2, 2 * QC], f16)
        nc.gpsimd.memset(dm_sb, 0.0)
        for g in range(2):
            for h in range(2):
                nc.gpsimd.affine_select(
                    out=dm_sb[:, g, h * QC:(h + 1) * QC],
                    in_=dm_sb[:, g, h * QC:(h + 1) * QC],
                    compare_op=is_gt,
                    fill=1.0,
                    base=P * (2 * g + h),
                    pattern=[[-1, QC]],
                    channel_multiplier=1,
                )
        mb_sb = const.tile([P, 2 * QC], f32)
        nc.gpsimd.memset(mb_sb, 0.0)
        for h in range(2):
            # complement of the live predicate (expr<=0) via a negated
            # iota, since only is_gt has codegen support
            nc.gpsimd.affine_select(
                out=mb_sb[:, h * QC:(h + 1) * QC],
                in_=mb_sb[:, h * QC:(h + 1) * QC],
                compare_op=is_gt,
                fill=-1.0e5,
                base=1 - P * (2 + h),
                pattern=[[1, QC]],
                channel_multiplier=-1,
            )
        # contiguous copy of the final trimmed pair's live-region mask so
        # the tail masking is a single tensor_tensor over [QC/2, QC+QC/4)
        mbt_sb = const.tile([P, 3 * QC // 4], f32)
        nc.vector.tensor_copy(mbt_sb[:, 0:QC // 2], mb_sb[:, QC // 2:QC])
        nc.vector.tensor_copy(
            mbt_sb[:, QC // 2:], mb_sb[:, 2 * QC - QC // 4:]
        )
        if not GMASK:
            dm8_sb = const.tile([P, 2 * QC], f8)
            nc.vector.tensor_copy(dm8_sb, dm_sb[:, 0, :])
        # tiny pass-through used to chain executions when benchmarking
        cht = const.tile([P, P], f32, name="cht")
        nc.scalar.dma_start(out=cht, in_=ch_h[:])
        nc.scalar.dma_start(out=cho_h[:], in_=cht)

        # ---- persistent activations ----
        qk_sb = pers.tile([P, S], f16)    # rows 0:64 Q^T, rows 64:128 K^T
        kt2_sb = pers.tile([64, S], f16)  # K^T relocated to partitions 0-63
        if ROWPACK_SCORES:
            qt2_sb = pers.tile([P, S], f16)  # rows 64:128 = Q^T duplicate
        v_sb = pers.tile([P, NT, VW], f16)  # V natural tiles + ones column
        nc.vector.memset(v_sb[:, :, 64:65], 1.0)
        # fp8 copy of V, DoubleRow-interleaved by tile pair (+ ones col):
        # steady-state (non-diagonal) PV runs as one fp8 DoubleRow matmul
        # per pair, halving its PE stream time.  Softmax averaging over
        # >=512 keys keeps the fp8 V/P noise ~0.2% on those pairs; the
        # few-key diagonal pairs stay on the fp16 path.
        v8_sb = pers.tile([P, NT // 2, 2, 80], f8)
        nc.vector.memset(v8_sb[:, :, :, 64:65], 1.0)

        xtc_tiles = {}
        po_tiles = {}

        def emit_xtc_dma(c):
            if c >= NQ:
                return
            t = xt_pool.tile([P, ND, QC], f16, name="xtc", tag="xtc")
            nc.sync.dma_start(out=t, in_=xt_h[:, c, :, :])
            xtc_tiles[c] = t

        def qk_store(c, ps_qk):
            """PSUM->SBUF move of packed Q^T/K^T, split across both
            elementwise engines so neither queue stalls long."""
            cs = slice(c * QC, (c + 1) * QC)
            h = QC // 2
            nc.scalar.copy(qk_sb[:, c * QC:c * QC + h], ps_qk[:, 0:h])
            nc.vector.tensor_copy(qk_sb[:, c * QC + h:(c + 1) * QC],
                                  ps_qk[:, h:])
            # relocations (partition shifts need a DMA, not a DVE op); on
            # the sync ring so the gpsimd queue stays free for the
            # latency-critical diagonal masks
            nc.sync.dma_start(out=kt2_sb[:, cs], in_=qk_sb[64:128, cs])
            if ROWPACK_SCORES:
                nc.sync.dma_start(out=qt2_sb[64:128, cs], in_=qk_sb[0:64, cs])

        def proj0_gen():
            """Chunk-0 projection (fp16)."""
            xtc = xtc_tiles.pop(0)
            ps_qk = ps_sm.tile([P, QC], f32, name="ps_qk", tag="sm")
            for dc in range(ND):
                nc.tensor.matmul(
                    ps_qk, lhsT=wqk_sb[:, dc, :], rhs=xtc[:, dc, :],
                    start=(dc == 0), stop=(dc == ND - 1),
                )
                if dc % 2 == 1:
                    yield
            qk_store(0, ps_qk)
            yield
            # V projection: two col-tiled halves into one PSUM tile
            ps_v = ps_sm.tile([P, QC // 2], f32, name="ps_v", tag="sm")
            for dc in range(ND):
                st, sp = (dc == 0), (dc == ND - 1)
                nc.tensor.matmul(
                    ps_v[0:64, :], lhsT=wv_sb[:, dc, :],
                    rhs=xtc[:, dc, 0:QC // 2], start=st, stop=sp,
                )
                nc.tensor.matmul(
                    ps_v[64:128, :], lhsT=wv_sb[:, dc, :],
                    rhs=xtc[:, dc, QC // 2:], start=st, stop=sp,
                    tile_position=(0, 64),
                )
                if dc % 4 == 3:
                    yield
            vt_sb = stage.tile([P, QC // 2], f16, name="vt_sb", tag="vt")
            nc.vector.tensor_copy(vt_sb, ps_v)
            yield
            yield from v_epilog(0, vt_sb)
            emit_xtc_dma(2)

        def v_epilog(c, vt_sb):
            """Shared V-projection tail: transposes -> v_sb/v8_sb tiles.
            The [128,128] transposes run on the DMA xbar engine instead of
            the PE, freeing PE stream time; the copies then read SBUF
            (faster DVE perf modes than PSUM sources)."""
            # transpose [128,128] once per half: rows 0:64 of the result
            # are t-tile 4c+k, rows 64:128 are t-tile 4c+2+k
            for k in range(2):
                ps_tv = ps_sm.tile([P, P], f16, name="ps_tv", tag="sm")
                nc.tensor.transpose(ps_tv, vt_sb[:, k * P:(k + 1) * P], ident16)
                src2 = ps_tv[:, :].rearrange("p (a b) -> p a b", a=2)
                # merged strided copies: one op covers both t-tiles
                nc.vector.tensor_copy(
                    v_sb[:, 4 * c + k:4 * c + k + 3:2, 0:64], src2
                )
                nc.vector.tensor_copy(
                    v8_sb[:, 2 * c:2 * c + 2, k, 0:64], src2
                )
                yield

        def qkproj_gen(c):
            """Q,K projection of chunk c>=1 (packed, M=128)."""
            xtc = xtc_tiles[c]
            ps_qk = ps_sm.tile([P, QC], f32, name="ps_qk", tag="sm")
            for dc in range(ND):
                nc.tensor.matmul(
                    ps_qk, lhsT=wqk_sb[:, dc, :], rhs=xtc[:, dc, :],
                    start=(dc == 0), stop=(dc == ND - 1),
                )
                if dc % 2 == 1:
                    yield
            qk_store(c, ps_qk)
            yield

        def vproj_gen(c):
            """V projection of chunk c>=1: two col-tiled halves."""
            xtc = xtc_tiles.pop(c)
            ps_v = ps_sm.tile([P, QC // 2], f32, name="ps_v", tag="sm")
            for dc in range(ND):
                st, sp = (dc == 0), (dc == ND - 1)
                nc.tensor.matmul(
                    ps_v[0:64, :], lhsT=wv_sb[:, dc, :],
                    rhs=xtc[:, dc, 0:QC // 2], start=st, stop=sp,
                )
                nc.tensor.matmul(
                    ps_v[64:128, :], lhsT=wv_sb[:, dc, :],
                    rhs=xtc[:, dc, QC // 2:], start=st, stop=sp,
                    tile_position=(0, 64),
                )
                if dc % 4 == 3:
                    yield
            vt_sb = stage.tile([P, QC // 2], f16, name="vt_sb", tag="vt")
            nc.vector.tensor_copy(vt_sb, ps_v)
            yield
            yield from v_epilog(c, vt_sb)
            emit_xtc_dma(c + 2)

        def emit_pair(c, jp, po):
            """Emit the score matmuls + exp (+ masks) for pair jp of chunk
            c.  Returns a closure that emits the PV matmul(s); the caller
            defers it by two pairs so the PE queue always has ready score
            work ahead of a PV that stalls on its exp -- an in-order queue
            would otherwise expose the PE pipeline-fill latency on every
            exp wait."""
            cs = slice(c * QC, (c + 1) * QC)
            njs = 4 * (c + 1)
            j0, j1 = 2 * jp, 2 * jp + 1
            jj = j0 - 4 * c
            trimmed = jj == 2  # second diagonal pair: >62% masked
            # pair 0 avoids the row-packed path so a fresh chunk's first
            # scores don't wait on the qt2 relocation DMA
            packed = ROWPACK_SCORES and jp > 0

            def ktlo(j):
                return kt2_sb[:, j * P:(j + 1) * P]

            def qthi(lo):
                return qt2_sb[64:128, c * QC + lo:(c + 1) * QC]

            ps_s = ps_pair.tile([P, 2 * QC], f32, name="ps_s", tag="pair")
            if trimmed or (jj == 0 and c == 0):
                pt = pt_pool.tile([P, 2 * QC], f16, name="pt", tag="pt")
            if trimmed:
                # jj=2 half: only q in [256,512) is live; jj=3 half: only q in
                # [384,512), remapped to columns [512,640) so one ACT covers a
                # contiguous [256,640) region.
                nc.tensor.matmul(
                    ps_s[:, QC // 2:QC],
                    lhsT=ktlo(j0),
                    rhs=qk_sb[0:64, c * QC + QC // 2:(c + 1) * QC],
                    start=True, stop=True,
                )
                nc.tensor.matmul(
                    ps_s[:, QC:QC + QC // 4],
                    lhsT=qk_sb[64:128, j1 * P:(j1 + 1) * P],
                    rhs=qthi(3 * QC // 4),
                    start=True, stop=True, tile_position=(64, 0),
                )
                nc.gpsimd.memset(pt[:, 0:QC // 2], 0.0)
                if c == NQ - 1:
                    # final pair: mask pre-exp (drained pipeline; shortens
                    # the serial ACT->mask->PV tail chain)
                    nc.vector.tensor_tensor(
                        ps_s[:, QC // 2:QC + QC // 4],
                        ps_s[:, QC // 2:QC + QC // 4], mbt_sb, op=add_op,
                    )
                    nc.scalar.activation(
                        pt[:, QC // 2:QC + QC // 4],
                        ps_s[:, QC // 2:QC + QC // 4], Exp, scale=SCALE,
                    )
                else:
                    nc.scalar.activation(
                        pt[:, QC // 2:QC + QC // 4],
                        ps_s[:, QC // 2:QC + QC // 4], Exp, scale=SCALE,
                    )
                    # causal trimming on GpSimd (both slices reduce to the
                    # same local predicate: live where col >= partition)
                    for off, w in ((QC // 2, QC // 2), (QC, QC // 4)):
                        nc.gpsimd.affine_select(
                            out=pt[:, off:off + w], in_=pt[:, off:off + w],
                            compare_op=is_gt, fill=0.0,
                            base=1, pattern=[[1, w]],
                            channel_multiplier=-1,
                        )
                def pv_trimmed():
                    nc.tensor.matmul(
                        po[:, 3 * QC // 4:], lhsT=v_sb[:, j1, 0:65],
                        rhs=pt[:, QC:QC + QC // 4], start=False, stop=False,
                    )
                    nc.tensor.matmul(
                        po, lhsT=v_sb[:, j0, 0:65], rhs=pt[:, 0:QC],
                        start=False, stop=(j1 == njs - 1),
                    )
                return pv_trimmed
            nc.tensor.matmul(
                ps_s[:, 0:QC],
                lhsT=ktlo(j0), rhs=qk_sb[0:64, cs],
                start=True, stop=True,
            )
            if packed:
                nc.tensor.matmul(
                    ps_s[:, QC:],
                    lhsT=qk_sb[64:128, j1 * P:(j1 + 1) * P],
                    rhs=qthi(0),
                    start=True, stop=True, tile_position=(64, 0),
                )
            else:
                nc.tensor.matmul(
                    ps_s[:, QC:],
                    lhsT=ktlo(j1), rhs=qk_sb[0:64, cs],
                    start=True, stop=True,
                )
            if jj == 0 and c == 0:  # chunk-0 diagonal pair: fp16 + mask
                nc.scalar.activation(pt, ps_s, Exp, scale=SCALE)
                nc.vector.tensor_tensor(pt, pt, dm_sb[:, 0, :], op=mult)

                def pv_c0():
                    nc.tensor.matmul(
                        po, lhsT=v_sb[:, j0, 0:65], rhs=pt[:, 0:QC],
                        start=(j0 == 0), stop=False,
                    )
                    nc.tensor.matmul(
                        po, lhsT=v_sb[:, j1, 0:65], rhs=pt[:, QC:],
                        start=False, stop=(j1 == njs - 1),
                    )
                return pv_c0
            # fp8 exp tile, flat [P, 2QC]; the PV matmul reads it through a
            # DoubleRow-interleaved [P, 2, QC] view
            pt8 = pt8_pool.tile([P, 2 * QC], f8, name="pt8", tag="pt8")
            diag = jj == 0  # c >= 1 here (c == 0 handled above)
            if SCHRAB:
                # split exp: ScalarE ACTIVATE on the first THETA columns,
                # DVE Schraudolph exp2 bit trick on the rest.  Both run
                # concurrently, halving the per-pair exp latency and
                # splitting the elementwise load across the two engines.
                th = QC if diag else THETA
                nc.scalar.activation(
                    pt8[:, 0:th], ps_s[:, 0:th], Exp, scale=SCALE
                )
                nc.vector.tensor_scalar(
                    pt8[:, th:].bitcast(i8), ps_s[:, th:],
                    SCHRA_A, SCHRA_B, op0=mult, op1=add_op,
                )
            else:
                nc.scalar.activation(pt8, ps_s, Exp, scale=SCALE)
            if diag:
                if GMASK:
                    # in-place causal zeroing on GpSimd: keeps where
                    # query >= key, fills 0 above the diagonal
                    for ko in range(2):
                        nc.gpsimd.affine_select(
                            out=pt8[:, ko * QC:(ko + 1) * QC],
                            in_=pt8[:, ko * QC:(ko + 1) * QC],
                            compare_op=is_gt, fill=0.0,
                            base=1 - P * ko,
                            pattern=[[1, QC]],
                            channel_multiplier=-1,
                        )
                else:
                    nc.vector.tensor_tensor(pt8, pt8, dm8_sb, op=mult)

            def pv_dr():
                nc.tensor.matmul(
                    po, lhsT=v8_sb[:, jp, :, 0:65],
                    rhs=pt8[:, :].rearrange("p (a b) -> p a b", a=2),
                    start=(j0 == 0), stop=(j1 == njs - 1),
                    perf_mode=DR,
                )
            return pv_dr

        def epi_gen(c):
            """Store U^T + denominators for chunk c; the host normalizes."""
            last = c == NQ - 1
            po = po_tiles.pop(c)
            osb = stage.tile([65, QC], f32, name="osb", tag="osb")
            nc.scalar.copy(osb[:, 0:QC // 2], po[:, 0:QC // 2])
            nc.vector.tensor_copy(osb[:, QC // 2:], po[:, QC // 2:])
            yield
            eng = nc.scalar if last else nc.sync
            eng.dma_start(out=o_h[c], in_=osb)
            yield

        # Software pipeline: a global queue of deferrable PE work
        # (projections of later chunks, epilogues of finished chunks) is
        # drained in small bursts between attention pairs, so the PE fills
        # its exp-wait slack and never idles across chunk boundaries.
        proj_pending = []   # [(due, generator)] sorted by deadline
        epi_pending = []    # generators (no deadline)

        def pull_one(max_due=None):
            while proj_pending:
                due, g = proj_pending[0]
                if max_due is not None and due > max_due:
                    break
                try:
                    next(g)
                    return
                except StopIteration:
                    proj_pending.pop(0)
            while epi_pending:
                try:
                    next(epi_pending[0])
                    return
                except StopIteration:
                    epi_pending.pop(0)

        def ensure(due):
            while proj_pending and proj_pending[0][0] <= due:
                _, g = proj_pending[0]
                for _ in g:
                    pass
                proj_pending.pop(0)

        xtc_tiles[0] = xtc0
        emit_xtc_dma(1)
        for _ in proj0_gen():
            pass
        # deadline-ordered deferred PE work: qkproj(c) is due at chunk-c
        # start, vproj(c) only at chunk c's first diagonal pair (pair 2c),
        # which spreads projection bursts across the ACT-paced pair loop
        for c in range(1, NQ):
            proj_pending.append(((c, 0), qkproj_gen(c)))
            proj_pending.append(((c, 1), vproj_gen(c)))
        proj_pending.sort(key=lambda t: t[0])
        for c in range(NQ):
            ensure((c, 0))
            po = ps_op.tile([65, QC], f32, name="po", tag="po")
            po_tiles[c] = po
            pv_queue = []
            for jp in range(2 * (c + 1)):
                if jp == 2 * c:
                    ensure((c, 1))
                pv_queue.append(emit_pair(c, jp, po))
                if len(pv_queue) > 2:
                    pv_queue.pop(0)()
                # front-load upcoming projections into the pair slack so
                # they (and the relocation DMAs) beat their deadlines
                pulls = 3 if jp < 4 else 1
                for _ in range(pulls):
                    pull_one(max_due=(c + 1, 0))
            while pv_queue:
                pv_queue.pop(0)()
            epi_pending.append(epi_gen(c))
        for _, g in proj_pending:
            for _ in g:
                pass
        proj_pending.clear()
        for g in epi_pending:
            for _ in g:
                pass
    nc.finalize()
    return nc


def _dev_w(w):
    # [D, m] -> [P, ND, m] with d = dc*P + p
    w = np.ascontiguousarray(w).astype(np.float16)
    return np.ascontiguousarray(w.reshape(ND, P, -1).transpose(1, 0, 2))


def _host_inputs(x, wQ, wK, wV):
    x = np.asarray(x, dtype=np.float32)
    wqk = _dev_w(np.concatenate([np.asarray(wQ).T, np.asarray(wK).T], axis=1))
    wv = _dev_w(np.asarray(wV).T)
    ww = np.ascontiguousarray(np.concatenate([wqk, wv], axis=2))
    chain = np.zeros((P, P), np.float32)
    in_maps = []
    for b in range(B):
        # x_b.T [D, S] -> [P, NQ, ND, QC] with d = dc*P + p, s = c*QC + sc
        xt = np.ascontiguousarray(
            x[b].T.astype(np.float16)
            .reshape(ND, P, NQ, QC)
            .transpose(1, 2, 0, 3)
        )
        in_maps.append({"xt": xt, "ww": ww, "chain": chain})
    return in_maps


def kernel(x, wQ, wK, wV):
    from concourse.bass_utils import run_bass_kernel_spmd

    if "nc" not in _CACHED:
        _CACHED["nc"] = _build_nc()
    nc = _CACHED["nc"]
    in_maps = _host_inputs(x, wQ, wK, wV)
    res = run_bass_kernel_spmd(nc, in_maps, core_ids=list(range(B)))
    out = np.empty((B, S, DK), np.float32)
    for b in range(B):
        u = res.results[b]["o"]          # [NQ, 65, QC]
        o = u[:, :64, :] / u[:, 64:65, :]
        out[b] = o.transpose(0, 2, 1).reshape(S, DK)
    return out


# revision 30
# speedup vs baseline: 1.1307x; 1.0355x over previous
"""Trainium2 Bass kernel for a single causal attention head.

Reference computation (per batch element b):
    Q = x_b @ wQ.T ; K = x_b @ wK.T ; V = x_b @ wV.T          [S, DK]
    P = softmax(causal_mask(Q @ K.T * d_model**-0.5))          [S, S]
    O = P @ V                                                  [S, DK]

Sharding: one batch element per NeuronCore (B == n_cores == 8).
Weights are replicated. No collectives needed.

Per-core device layout (host pre-transposes/casts for PE-friendly fp16):
    xt    [D, S]   fp16  x_b transposed (contraction dim D on partitions)
    wqk   [D, 128] fp16  [wQ.T | wK.T]  -> packed projection, M=128
    wv    [D, 64]  fp16  wV.T
Output:
    o     [NQ, 65, QC] fp32  unnormalized U^T rows 0:64 + softmax
                             denominators in row 64; the host divides and
                             transposes during the gather/unshard step
                             (flash-attention partial-merge convention).

Device pipeline per 512-wide q-chunk c (fp16 matmuls, fp32 PSUM):
  - DMA xt chunk; project Q^T,K^T packed (M=128); project V^T with
    col-tiled half-chunks into one PSUM tile.
  - Duplicate Q^T/K^T across both partition halves (SBUF->SBUF DMA) so
    score matmuls for t-tiles j,j+1 run row-packed (rows 0-63 / 64-127
    of the PE array concurrently), writing one [128,1024] PSUM pair.
  - exp: the ScalarE ACTIVATE is the kernel bottleneck, so the exp work
    is split across two engines: even steady pairs use the ScalarE exp
    ACTIVATE -> fp8; odd steady pairs compute exp on the Vector engine
    with the Schraudolph exp2 bit trick (int8(z*8/ln2 + bias) bitcast
    to fp8e4m3 ~= 2^z with ~3% multiplicative ripple that washes out in
    the softmax average).
  - Diagonal pairs: ScalarE exp -> fp8, causal-masked in place by
    GpSimd affine_selects (keeps the Vector engine free).
  - PV: accumulate [V_j | 1].T @ P~_j into PSUM (U^T rows 0-63,
    softmax denominators in row 64) with fp8 DoubleRow matmuls.
"""

import numpy as np
import ml_dtypes

B, S, D, DK = 8, 4096, 1024, 64
P = 128
QC = 512          # q-chunk width (matmul moving dim)
NQ = S // QC      # 8 q-chunks
ND = D // P       # 8 contraction chunks
NT = S // P       # 32 t-tiles
SCALE = float(D) ** -0.5   # 1/32
VW = 66           # per-t-tile stride in v_sb (64 V cols + 1 ones + pad)

# Schraudolph exp2-bit-trick constants for the DVE exp path:
# i8 = trunc(score * SCHRA_A + SCHRA_B); bitcast(i8) as fp8e4m3 ~ exp(score/32)
SCHRA_A = SCALE * 8.0 * 1.4426950408889634
SCHRA_B = 55.95

# optimization knobs (validated on HW; flip off if a variant fails)
ROWPACK_SCORES = True   # row-packed score matmul pairs
SCHRAB = True           # split each pair's exp: ScalarE half + DVE half
GMASK = True            # diagonal-pair causal masks on GpSimd
THETA = 512             # exp split point: ScalarE gets [0:THETA) of 1024

_CACHED = {}


def _build_nc():
    import concourse.mybir as mybir
    import concourse.tile as tile
    from concourse import bacc
    from concourse.masks import make_identity
    from contextlib import ExitStack

    f32 = mybir.dt.float32
    f16 = mybir.dt.float16
    f8 = mybir.dt.float8e4
    i8 = mybir.dt.int8
    DR = mybir.MatmulPerfMode.DoubleRow
    Exp = mybir.ActivationFunctionType.Exp
    add_op = mybir.AluOpType.add
    mult = mybir.AluOpType.mult
    is_gt = mybir.AluOpType.is_gt

    nc = bacc.Bacc()
    # host pre-layouts: per-partition contiguous so DMAs need no gather.
    # ww packs [wqk | wv] along the last dim -> one DMA issue.
    WW = P + DK
    xt_h = nc.declare_dram_parameter("xt", [P, NQ, ND, QC], f16, isOutput=False)
    ww_h = nc.declare_dram_parameter("ww", [P, ND, WW], f16, isOutput=False)
    ch_h = nc.declare_dram_parameter("chain", [P, P], f32, isOutput=False)
    o_h = nc.declare_dram_parameter("o", [NQ, 65, QC], f32, isOutput=True)
    cho_h = nc.declare_dram_parameter("chain_o", [P, P], f32, isOutput=True)

    with tile.TileContext(nc) as tc, ExitStack() as ctx:
        const = ctx.enter_context(tc.tile_pool(name="const", bufs=1))
        xt_pool = ctx.enter_context(tc.tile_pool(name="xtp", bufs=2))
        pers = ctx.enter_context(tc.tile_pool(name="pers", bufs=1))
        pt_pool = ctx.enter_context(tc.tile_pool(name="ptp", bufs=3))
        pt8_pool = ctx.enter_context(tc.tile_pool(name="ptp8", bufs=6))
        stage = ctx.enter_context(tc.tile_pool(name="stage", bufs=2))
        # PSUM budget (8 banks): pair pool 3x2 + po 1 + sm 1 = 8.  Three
        # score buffers let the PE run up to three pairs ahead of the exp
        # engines, hiding per-matmul pipeline latency and exp jitter.
        ps_pair = ctx.enter_context(tc.tile_pool(name="ps_pair", bufs=3, space="PSUM"))
        ps_op = ctx.enter_context(tc.tile_pool(name="ps_op", bufs=1, space="PSUM"))
        ps_sm = ctx.enter_context(tc.tile_pool(name="ps_sm", bufs=1, space="PSUM"))

        # ---- weights first (small, gate the first projection), then the
        # first x chunk in quarters so the projection starts on the first
        # quarter while the rest streams in ----
        ww_sb = const.tile([P, ND, WW], f16)
        nc.sync.dma_start(out=ww_sb, in_=ww_h[:])
        wqk_sb = ww_sb[:, :, 0:P]
        wv_sb = ww_sb[:, :, P:]
        xtc0 = xt_pool.tile([P, ND, QC], f16, name="xtc", tag="xtc")
        for h in range(4):
            nc.sync.dma_start(
                out=xtc0[:, 2 * h:2 * (h + 1), :],
                in_=xt_h[:, 0, 2 * h:2 * (h + 1), :],
            )
        # ---- PE warm-up: dummy matmuls during the initial DMA wait so the
        # HAM clock gate is already at full rate (2.4 GHz) when the first
        # projection runs.  Depends only on one vector memset. ----
        warm_sb = const.tile([P, QC], f16, name="warm_sb")
        nc.vector.memset(warm_sb, 0.0)
        # warm in the (still unused) pair pool so the warmup drain never
        # blocks the first projection's ps_sm bank
        ps_warm = ps_pair.tile([P, 2 * QC], f32, name="ps_warm", tag="pair")
        for _ in range(3):
            nc.tensor.matmul(ps_warm[:, 0:QC], lhsT=warm_sb[:, 0:P],
                             rhs=warm_sb, start=True, stop=True)
        # identity before the causal masks: ident16 gates the first V
        # transpose (PE critical path)
        ident16 = const.tile([P, P], f16)
        make_identity(nc, ident16)
        # causal masks: fp16 multiplicative (post-exp) for the fp16-path
        # pairs, plus an fp32 additive bias (pre-exp) used only on the final
        # trimmed pair where it shortens the drained tail chain
        dm_sb = const.tile([P, 2, 2 * QC], f16)
        nc.gpsimd.memset(dm_sb, 0.0)
        for g in range(2):
            for h in range(2):
                nc.gpsimd.affine_select(
                    out=dm_sb[:, g, h * QC:(h + 1) * QC],
                    in_=dm_sb[:, g, h * QC:(h + 1) * QC],
                    compare_op=is_gt,
                    fill=1.0,
                    base=P * (2 * g + h),
                    pattern=[[-1, QC]],
                    channel_multiplier=1,
                )
        mb_sb = const.tile([P, 2 * QC], f32)
        nc.gpsimd.memset(mb_sb, 0.0)
        for h in range(2):
            # complement of the live predicate (expr<=0) via a negated
            # iota, since only is_gt has codegen support
            nc.gpsimd.affine_select(
                out=mb_sb[:, h * QC:(h + 1) * QC],
                in_=mb_sb[:, h * QC:(h + 1) * QC],
                compare_op=is_gt,
                fill=-1.0e5,
                base=1 - P * (2 + h),
                pattern=[[1, QC]],
                channel_multiplier=-1,
            )
        # contiguous copy of the final trimmed pair's live-region mask so
        # the tail masking is a single tensor_tensor over [QC/2, QC+QC/4)
        mbt_sb = const.tile([P, 3 * QC // 4], f32)
        nc.vector.tensor_copy(mbt_sb[:, 0:QC // 2], mb_sb[:, QC // 2:QC])
        nc.vector.tensor_copy(
            mbt_sb[:, QC // 2:], mb_sb[:, 2 * QC - QC // 4:]
        )
        if not GMASK:
            dm8_sb = const.tile([P, 2 * QC], f8)
            nc.vector.tensor_copy(dm8_sb, dm_sb[:, 0, :])
        # tiny pass-through used to chain executions when benchmarking
        cht = const.tile([P, P], f32, name="cht")
        nc.scalar.dma_start(out=cht, in_=ch_h[:])
        nc.scalar.dma_start(out=cho_h[:], in_=cht)

        # ---- persistent activations ----
        qk_sb = pers.tile([P, S], f16)    # rows 0:64 Q^T, rows 64:128 K^T
        kt2_sb = pers.tile([64, S], f16)  # K^T relocated to partitions 0-63
        if ROWPACK_SCORES:
            qt2_sb = pers.tile([P, S], f16)  # rows 64:128 = Q^T duplicate
        v_sb = pers.tile([P, NT, VW], f16)  # V natural tiles + ones column
        nc.vector.memset(v_sb[:, :, 64:65], 1.0)
        # fp8 copy of V, DoubleRow-interleaved by tile pair (+ ones col):
        # steady-state (non-diagonal) PV runs as one fp8 DoubleRow matmul
        # per pair, halving its PE stream time.  Softmax averaging over
        # >=512 keys keeps the fp8 V/P noise ~0.2% on those pairs; the
        # few-key diagonal pairs stay on the fp16 path.
        v8_sb = pers.tile([P, NT // 2, 2, 80], f8)
        nc.vector.memset(v8_sb[:, :, :, 64:65], 1.0)

        xtc_tiles = {}
        po_tiles = {}

        def emit_xtc_dma(c):
            if c >= NQ:
                return
            t = xt_pool.tile([P, ND, QC], f16, name="xtc", tag="xtc")
            nc.sync.dma_start(out=t, in_=xt_h[:, c, :, :])
            xtc_tiles[c] = t

        def qk_store(c, ps_qk):
            """PSUM->SBUF move of packed Q^T/K^T on ScalarE (the DVE queue
            carries the latency-critical Schraudolph exp halves)."""
            cs = slice(c * QC, (c + 1) * QC)
            nc.scalar.copy(qk_sb[:, cs], ps_qk)
            # relocations (partition shifts need a DMA, not a DVE op); on
            # the sync ring so the gpsimd queue stays free for the
            # latency-critical diagonal masks
            nc.sync.dma_start(out=kt2_sb[:, cs], in_=qk_sb[64:128, cs])
            if ROWPACK_SCORES:
                nc.sync.dma_start(out=qt2_sb[64:128, cs], in_=qk_sb[0:64, cs])

        def proj0_gen():
            """Chunk-0 projection (fp16)."""
            xtc = xtc_tiles.pop(0)
            ps_qk = ps_sm.tile([P, QC], f32, name="ps_qk", tag="sm")
            for dc in range(ND):
                nc.tensor.matmul(
                    ps_qk, lhsT=wqk_sb[:, dc, :], rhs=xtc[:, dc, :],
                    start=(dc == 0), stop=(dc == ND - 1),
                )
                if dc % 2 == 1:
                    yield
            qk_store(0, ps_qk)
            yield
            # V projection: two col-tiled halves into one PSUM tile
            ps_v = ps_sm.tile([P, QC // 2], f32, name="ps_v", tag="sm")
            for dc in range(ND):
                st, sp = (dc == 0), (dc == ND - 1)
                nc.tensor.matmul(
                    ps_v[0:64, :], lhsT=wv_sb[:, dc, :],
                    rhs=xtc[:, dc, 0:QC // 2], start=st, stop=sp,
                )
                nc.tensor.matmul(
                    ps_v[64:128, :], lhsT=wv_sb[:, dc, :],
                    rhs=xtc[:, dc, QC // 2:], start=st, stop=sp,
                    tile_position=(0, 64),
                )
                if dc % 4 == 3:
                    yield
            vt_sb = stage.tile([P, QC // 2], f16, name="vt_sb", tag="vt")
            nc.vector.tensor_copy(vt_sb, ps_v)
            yield
            yield from v_epilog(0, vt_sb)
            emit_xtc_dma(2)

        def v_epilog(c, vt_sb):
            """Shared V-projection tail: transposes -> v_sb/v8_sb tiles.
            The [128,128] transposes run on the DMA xbar engine instead of
            the PE, freeing PE stream time; the copies then read SBUF
            (faster DVE perf modes than PSUM sources)."""
            # transpose [128,128] once per half: rows 0:64 of the result
            # are t-tile 4c+k, rows 64:128 are t-tile 4c+2+k
            for k in range(2):
                ps_tv = ps_sm.tile([P, P], f16, name="ps_tv", tag="sm")
                nc.tensor.transpose(ps_tv, vt_sb[:, k * P:(k + 1) * P], ident16)
                src2 = ps_tv[:, :].rearrange("p (a b) -> p a b", a=2)
                # merged strided copies: one op covers both t-tiles
                nc.vector.tensor_copy(
                    v_sb[:, 4 * c + k:4 * c + k + 3:2, 0:64], src2
                )
                nc.vector.tensor_copy(
                    v8_sb[:, 2 * c:2 * c + 2, k, 0:64], src2
                )
                yield

        def qkproj_gen(c):
            """Q,K projection of chunk c>=1 (packed, M=128)."""
            xtc = xtc_tiles[c]
            ps_qk = ps_sm.tile([P, QC], f32, name="ps_qk", tag="sm")
            for dc in range(ND):
                nc.tensor.matmul(
                    ps_qk, lhsT=wqk_sb[:, dc, :], rhs=xtc[:, dc, :],
                    start=(dc == 0), stop=(dc == ND - 1),
                )
                if dc % 2 == 1:
                    yield
            qk_store(c, ps_qk)
            yield

        def vproj_gen(c):
            """V projection of chunk c>=1: two col-tiled halves."""
            xtc = xtc_tiles.pop(c)
            ps_v = ps_sm.tile([P, QC // 2], f32, name="ps_v", tag="sm")
            for dc in range(ND):
                st, sp = (dc == 0), (dc == ND - 1)
                nc.tensor.matmul(
                    ps_v[0:64, :], lhsT=wv_sb[:, dc, :],
                    rhs=xtc[:, dc, 0:QC // 2], start=st, stop=sp,
                )
                nc.tensor.matmul(
                    ps_v[64:128, :], lhsT=wv_sb[:, dc, :],
                    rhs=xtc[:, dc, QC // 2:], start=st, stop=sp,
                    tile_position=(0, 64),
                )
                if dc % 4 == 3:
                    yield
            vt_sb = stage.tile([P, QC // 2], f16, name="vt_sb", tag="vt")
            nc.vector.tensor_copy(vt_sb, ps_v)
            yield
            yield from v_epilog(c, vt_sb)
            emit_xtc_dma(c + 2)

        def emit_pair(c, jp, po):
            """Emit the score matmuls + exp (+ masks) for pair jp of chunk
            c.  Returns a closure that emits the PV matmul(s); the caller
            defers it by two pairs so the PE queue always has ready score
            work ahead of a PV that stalls on its exp -- an in-order queue
            would otherwise expose the PE pipeline-fill latency on every
            exp wait."""
            cs = slice(c * QC, (c + 1) * QC)
            njs = 4 * (c + 1)
            j0, j1 = 2 * jp, 2 * jp + 1
            jj = j0 - 4 * c
            trimmed = jj == 2  # second diagonal pair: >62% masked
            # pair 0 avoids the row-packed path so a fresh chunk's first
            # scores don't wait on the qt2 relocation DMA
            packed = ROWPACK_SCORES and jp > 0

            def ktlo(j):
                return kt2_sb[:, j * P:(j + 1) * P]

            def qthi(lo):
                return qt2_sb[64:128, c * QC + lo:(c + 1) * QC]

            ps_s = ps_pair.tile([P, 2 * QC], f32, name="ps_s", tag="pair")
            if trimmed or (jj == 0 and c == 0):
                pt = pt_pool.tile([P, 2 * QC], f16, name="pt", tag="pt")
            if trimmed:
                # jj=2 half: only q in [256,512) is live; jj=3 half: only q in
                # [384,512), remapped to columns [512,640) so one ACT covers a
                # contiguous [256,640) region.
                nc.tensor.matmul(
                    ps_s[:, QC // 2:QC],
                    lhsT=ktlo(j0),
                    rhs=qk_sb[0:64, c * QC + QC // 2:(c + 1) * QC],
                    start=True, stop=True,
                )
                nc.tensor.matmul(
                    ps_s[:, QC:QC + QC // 4],
                    lhsT=qk_sb[64:128, j1 * P:(j1 + 1) * P],
                    rhs=qthi(3 * QC // 4),
                    start=True, stop=True, tile_position=(64, 0),
                )
                nc.gpsimd.memset(pt[:, 0:QC // 2], 0.0)
                if c == NQ - 1:
                    # final pair: mask pre-exp (drained pipeline; shortens
                    # the serial ACT->mask->PV tail chain)
                    nc.vector.tensor_tensor(
                        ps_s[:, QC // 2:QC + QC // 4],
                        ps_s[:, QC // 2:QC + QC // 4], mbt_sb, op=add_op,
                    )
                    nc.scalar.activation(
                        pt[:, QC // 2:QC + QC // 4],
                        ps_s[:, QC // 2:QC + QC // 4], Exp, scale=SCALE,
                    )
                else:
                    nc.scalar.activation(
                        pt[:, QC // 2:QC + QC // 4],
                        ps_s[:, QC // 2:QC + QC // 4], Exp, scale=SCALE,
                    )
                    # causal trimming on GpSimd (both slices reduce to the
                    # same local predicate: live where col >= partition)
                    for off, w in ((QC // 2, QC // 2), (QC, QC // 4)):
                        nc.gpsimd.affine_select(
                            out=pt[:, off:off + w], in_=pt[:, off:off + w],
                            compare_op=is_gt, fill=0.0,
                            base=1, pattern=[[1, w]],
                            channel_multiplier=-1,
                        )
                def pv_trimmed():
                    nc.tensor.matmul(
                        po[:, 3 * QC // 4:], lhsT=v_sb[:, j1, 0:65],
                        rhs=pt[:, QC:QC + QC // 4], start=False, stop=False,
                    )
                    nc.tensor.matmul(
                        po, lhsT=v_sb[:, j0, 0:65], rhs=pt[:, 0:QC],
                        start=False, stop=(j1 == njs - 1),
                    )
                return pv_trimmed
            nc.tensor.matmul(
                ps_s[:, 0:QC],
                lhsT=ktlo(j0), rhs=qk_sb[0:64, cs],
                start=True, stop=True,
            )
            if packed:
                nc.tensor.matmul(
                    ps_s[:, QC:],
                    lhsT=qk_sb[64:128, j1 * P:(j1 + 1) * P],
                    rhs=qthi(0),
                    start=True, stop=True, tile_position=(64, 0),
                )
            else:
                nc.tensor.matmul(
                    ps_s[:, QC:],
                    lhsT=ktlo(j1), rhs=qk_sb[0:64, cs],
                    start=True, stop=True,
                )
            if jj == 0 and c == 0:  # chunk-0 diagonal pair: fp16 + mask
                nc.scalar.activation(pt, ps_s, Exp, scale=SCALE)
                nc.vector.tensor_tensor(pt, pt, dm_sb[:, 0, :], op=mult)

                def pv_c0():
                    nc.tensor.matmul(
                        po, lhsT=v_sb[:, j0, 0:65], rhs=pt[:, 0:QC],
                        start=(j0 == 0), stop=False,
                    )
                    nc.tensor.matmul(
                        po, lhsT=v_sb[:, j1, 0:65], rhs=pt[:, QC:],
                        start=False, stop=(j1 == njs - 1),
                    )
                return pv_c0
            # fp8 exp tile, flat [P, 2QC]; the PV matmul reads it through a
            # DoubleRow-interleaved [P, 2, QC] view
            pt8 = pt8_pool.tile([P, 2 * QC], f8, name="pt8", tag="pt8")
            diag = jj == 0  # c >= 1 here (c == 0 handled above)
            if SCHRAB:
                # split exp: ScalarE ACTIVATE on the first THETA columns,
                # DVE Schraudolph exp2 bit trick on the rest.  Both run
                # concurrently, halving the per-pair exp latency and
                # splitting the elementwise load across the two engines.
                th = QC if diag else THETA
                nc.scalar.activation(
                    pt8[:, 0:th], ps_s[:, 0:th], Exp, scale=SCALE
                )
                nc.vector.tensor_scalar(
                    pt8[:, th:].bitcast(i8), ps_s[:, th:],
                    SCHRA_A, SCHRA_B, op0=mult, op1=add_op,
                )
            else:
                nc.scalar.activation(pt8, ps_s, Exp, scale=SCALE)
            if diag:
                if GMASK:
                    # in-place causal zeroing on GpSimd: keeps where
                    # query >= key, fills 0 above the diagonal
                    for ko in range(2):
                        nc.gpsimd.affine_select(
                            out=pt8[:, ko * QC:(ko + 1) * QC],
                            in_=pt8[:, ko * QC:(ko + 1) * QC],
                            compare_op=is_gt, fill=0.0,
                            base=1 - P * ko,
                            pattern=[[1, QC]],
                            channel_multiplier=-1,
                        )
                else:
                    nc.vector.tensor_tensor(pt8, pt8, dm8_sb, op=mult)

            def pv_dr():
                nc.tensor.matmul(
                    po, lhsT=v8_sb[:, jp, :, 0:65],
                    rhs=pt8[:, :].rearrange("p (a b) -> p a b", a=2),
                    start=(j0 == 0), stop=(j1 == njs - 1),
                    perf_mode=DR,
                )
            return pv_dr

        def epi_gen(c):
            """Store U^T + denominators for chunk c; the host normalizes."""
            last = c == NQ - 1
            po = po_tiles.pop(c)
            osb = stage.tile([65, QC], f32, name="osb", tag="osb")
            nc.scalar.copy(osb, po)
            yield
            eng = nc.scalar if last else nc.sync
            eng.dma_start(out=o_h[c], in_=osb)
            yield

        # Software pipeline: a global queue of deferrable PE work
        # (projections of later chunks, epilogues of finished chunks) is
        # drained in small bursts between attention pairs, so the PE fills
        # its exp-wait slack and never idles across chunk boundaries.
        proj_pending = []   # [(due, generator)] sorted by deadline
        epi_pending = []    # generators (no deadline)

        def pull_one(max_due=None):
            while proj_pending:
                due, g = proj_pending[0]
                if max_due is not None and due > max_due:
                    break
                try:
                    next(g)
                    return
                except StopIteration:
                    proj_pending.pop(0)
            while epi_pending:
                try:
                    next(epi_pending[0])
                    return
                except StopIteration:
                    epi_pending.pop(0)

        def ensure(due):
            while proj_pending and proj_pending[0][0] <= due:
                _, g = proj_pending[0]
                for _ in g:
                    pass
                proj_pending.pop(0)

        xtc_tiles[0] = xtc0
        emit_xtc_dma(1)
        for _ in proj0_gen():
            pass
        # deadline-ordered deferred PE work: qkproj(c) is due at chunk-c
        # start, vproj(c) only at chunk c's first diagonal pair (pair 2c),
        # which spreads projection bursts across the ACT-paced pair loop
        for c in range(1, NQ):
            proj_pending.append(((c, 0), qkproj_gen(c)))
            proj_pending.append(((c, 1), vproj_gen(c)))
        proj_pending.sort(key=lambda t: t[0])
        for c in range(NQ):
            ensure((c, 0))
            po = ps_op.tile([65, QC], f32, name="po", tag="po")
            po_tiles[c] = po
            pv_queue = []
            for jp in range(2 * (c + 1)):
                if jp == 2 * c:
                    ensure((c, 1))
                pv_queue.append(emit_pair(c, jp, po))
                if len(pv_queue) > 2:
                    pv_queue.pop(0)()
                # front-load upcoming projections into the pair slack so
                # they (and the relocation DMAs) beat their deadlines
                pulls = 3 if jp < 4 else 1
                for _ in range(pulls):
                    pull_one(max_due=(c + 1, 0))
            while pv_queue:
                pv_queue.pop(0)()
            epi_pending.append(epi_gen(c))
        for _, g in proj_pending:
            for _ in g:
                pass
        proj_pending.clear()
        for g in epi_pending:
            for _ in g:
                pass
    nc.finalize()
    return nc


def _dev_w(w):
    # [D, m] -> [P, ND, m] with d = dc*P + p
    w = np.ascontiguousarray(w).astype(np.float16)
    return np.ascontiguousarray(w.reshape(ND, P, -1).transpose(1, 0, 2))


def _host_inputs(x, wQ, wK, wV):
    x = np.asarray(x, dtype=np.float32)
    wqk = _dev_w(np.concatenate([np.asarray(wQ).T, np.asarray(wK).T], axis=1))
    wv = _dev_w(np.asarray(wV).T)
    ww = np.ascontiguousarray(np.concatenate([wqk, wv], axis=2))
    chain = np.zeros((P, P), np.float32)
    in_maps = []
    for b in range(B):
        # x_b.T [D, S] -> [P, NQ, ND, QC] with d = dc*P + p, s = c*QC + sc
        xt = np.ascontiguousarray(
            x[b].T.astype(np.float16)
            .reshape(ND, P, NQ, QC)
            .transpose(1, 2, 0, 3)
        )
        in_maps.append({"xt": xt, "ww": ww, "chain": chain})
    return in_maps


def kernel(x, wQ, wK, wV):
    from concourse.bass_utils import run_bass_kernel_spmd

    if "nc" not in _CACHED:
        _CACHED["nc"] = _build_nc()
    nc = _CACHED["nc"]
    in_maps = _host_inputs(x, wQ, wK, wV)
    res = run_bass_kernel_spmd(nc, in_maps, core_ids=list(range(B)))
    out = np.empty((B, S, DK), np.float32)
    for b in range(B):
        u = res.results[b]["o"]          # [NQ, 65, QC]
        o = u[:, :64, :] / u[:, 64:65, :]
        out[b] = o.transpose(0, 2, 1).reshape(S, DK)
    return out


# revision 32
# speedup vs baseline: 1.1524x; 1.0192x over previous
"""Trainium2 Bass kernel for a single causal attention head.

Reference computation (per batch element b):
    Q = x_b @ wQ.T ; K = x_b @ wK.T ; V = x_b @ wV.T          [S, DK]
    P = softmax(causal_mask(Q @ K.T * d_model**-0.5))          [S, S]
    O = P @ V                                                  [S, DK]

Sharding: one batch element per NeuronCore (B == n_cores == 8).
Weights are replicated. No collectives needed.

Per-core device layout (host pre-transposes/casts for PE-friendly fp16):
    xt    [D, S]   fp16  x_b transposed (contraction dim D on partitions)
    wqk   [D, 128] fp16  [wQ.T | wK.T]  -> packed projection, M=128
    wv    [D, 64]  fp16  wV.T
Output:
    o     [NQ, 65, QC] fp32  unnormalized U^T rows 0:64 + softmax
                             denominators in row 64; the host divides and
                             transposes during the gather/unshard step
                             (flash-attention partial-merge convention).

Device pipeline per 512-wide q-chunk c (fp16 matmuls, fp32 PSUM):
  - DMA xt chunk; project Q^T,K^T packed (M=128); project V^T with
    col-tiled half-chunks into one PSUM tile.
  - Duplicate Q^T/K^T across both partition halves (SBUF->SBUF DMA) so
    score matmuls for t-tiles j,j+1 run row-packed (rows 0-63 / 64-127
    of the PE array concurrently), writing one [128,1024] PSUM pair.
  - exp: the ScalarE ACTIVATE is the kernel bottleneck, so the exp work
    is split across two engines: even steady pairs use the ScalarE exp
    ACTIVATE -> fp8; odd steady pairs compute exp on the Vector engine
    with the Schraudolph exp2 bit trick (int8(z*8/ln2 + bias) bitcast
    to fp8e4m3 ~= 2^z with ~3% multiplicative ripple that washes out in
    the softmax average).
  - Diagonal pairs: ScalarE exp -> fp8, causal-masked in place by
    GpSimd affine_selects (keeps the Vector engine free).
  - PV: accumulate [V_j | 1].T @ P~_j into PSUM (U^T rows 0-63,
    softmax denominators in row 64) with fp8 DoubleRow matmuls.
"""

import numpy as np

B, S, D, DK = 8, 4096, 1024, 64
P = 128
QC = 512          # q-chunk width (matmul moving dim)
NQ = S // QC      # 8 q-chunks
ND = D // P       # 8 contraction chunks
NT = S // P       # 32 t-tiles
SCALE = float(D) ** -0.5   # 1/32
VW = 66           # per-t-tile stride in v_sb (64 V cols + 1 ones + pad)

# Schraudolph exp2-bit-trick constants for the DVE exp path:
# i8 = trunc(score * SCHRA_A + SCHRA_B); bitcast(i8) as fp8e4m3 ~ exp(score/32)
SCHRA_A = SCALE * 8.0 * 1.4426950408889634
SCHRA_B = 55.95

# optimization knobs (validated on HW; flip off if a variant fails)
ROWPACK_SCORES = True   # row-packed score matmul pairs
SCHRAB = True           # split each pair's exp: ScalarE half + DVE half
GMASK = True            # diagonal-pair causal masks on GpSimd
THETA = 512             # exp split point: ScalarE gets [0:THETA) of 1024

_CACHED = {}


def _build_nc():
    import concourse.mybir as mybir
    import concourse.tile as tile
    from concourse import bacc
    from concourse.masks import make_identity
    from contextlib import ExitStack

    f32 = mybir.dt.float32
    f16 = mybir.dt.float16
    f8 = mybir.dt.float8e4
    i8 = mybir.dt.int8
    DR = mybir.MatmulPerfMode.DoubleRow
    Exp = mybir.ActivationFunctionType.Exp
    add_op = mybir.AluOpType.add
    mult = mybir.AluOpType.mult
    is_gt = mybir.AluOpType.is_gt

    nc = bacc.Bacc()
    # host pre-layouts: per-partition contiguous so DMAs need no gather.
    # ww packs [wqk | wv] along the last dim -> one DMA issue.
    WW = P + DK
    xt_h = nc.declare_dram_parameter("xt", [P, NQ, ND, QC], f16, isOutput=False)
    ww_h = nc.declare_dram_parameter("ww", [P, ND, WW], f16, isOutput=False)
    ch_h = nc.declare_dram_parameter("chain", [P, P], f32, isOutput=False)
    o_h = nc.declare_dram_parameter("o", [NQ, 65, QC], f32, isOutput=True)
    cho_h = nc.declare_dram_parameter("chain_o", [P, P], f32, isOutput=True)

    with tile.TileContext(nc) as tc, ExitStack() as ctx:
        const = ctx.enter_context(tc.tile_pool(name="const", bufs=1))
        xt_pool = ctx.enter_context(tc.tile_pool(name="xtp", bufs=2))
        pers = ctx.enter_context(tc.tile_pool(name="pers", bufs=1))
        pt_pool = ctx.enter_context(tc.tile_pool(name="ptp", bufs=3))
        pt8_pool = ctx.enter_context(tc.tile_pool(name="ptp8", bufs=6))
        stage = ctx.enter_context(tc.tile_pool(name="stage", bufs=2))
        # PSUM budget (8 banks): pair pool 3x2 + po 1 + sm 1 = 8.  Three
        # score buffers let the PE run up to three pairs ahead of the exp
        # engines, hiding per-matmul pipeline latency and exp jitter.
        ps_pair = ctx.enter_context(tc.tile_pool(name="ps_pair", bufs=3, space="PSUM"))
        ps_op = ctx.enter_context(tc.tile_pool(name="ps_op", bufs=1, space="PSUM"))
        ps_sm = ctx.enter_context(tc.tile_pool(name="ps_sm", bufs=1, space="PSUM"))

        # ---- weights first (small, gate the first projection), then the
        # first x chunk in quarters so the projection starts on the first
        # quarter while the rest streams in ----
        ww_sb = const.tile([P, ND, WW], f16)
        nc.sync.dma_start(out=ww_sb, in_=ww_h[:])
        wqk_sb = ww_sb[:, :, 0:P]
        wv_sb = ww_sb[:, :, P:]
        xtc0 = xt_pool.tile([P, ND, QC], f16, name="xtc", tag="xtc")
        for h in range(4):
            nc.sync.dma_start(
                out=xtc0[:, 2 * h:2 * (h + 1), :],
                in_=xt_h[:, 0, 2 * h:2 * (h + 1), :],
            )
        # ---- PE warm-up: dummy matmuls during the initial DMA wait so the
        # HAM clock gate is already at full rate (2.4 GHz) when the first
        # projection runs.  Depends only on one vector memset. ----
        warm_sb = const.tile([P, QC], f16, name="warm_sb")
        nc.vector.memset(warm_sb, 0.0)
        ps_warm = ps_sm.tile([P, QC], f32, name="ps_warm", tag="sm")
        for _ in range(3):
            nc.tensor.matmul(ps_warm, lhsT=warm_sb[:, 0:P], rhs=warm_sb,
                             start=True, stop=True)
        # identity before the causal masks: ident16 gates the first V
        # transpose (PE critical path)
        ident16 = const.tile([P, P], f16)
        make_identity(nc, ident16)
        # causal masks: fp16 multiplicative (post-exp) for the fp16-path
        # pairs, plus an fp32 additive bias (pre-exp) used only on the final
        # trimmed pair where it shortens the drained tail chain
        dm_sb = const.tile([P, 2, 2 * QC], f16)
        nc.gpsimd.memset(dm_sb, 0.0)
        for g in range(2):
            for h in range(2):
                nc.gpsimd.affine_select(
                    out=dm_sb[:, g, h * QC:(h + 1) * QC],
                    in_=dm_sb[:, g, h * QC:(h + 1) * QC],
                    compare_op=is_gt,
                    fill=1.0,
                    base=P * (2 * g + h),
                    pattern=[[-1, QC]],
                    channel_multiplier=1,
                )
        mb_sb = const.tile([P, 2 * QC], f32)
        nc.gpsimd.memset(mb_sb, 0.0)
        for h in range(2):
            # complement of the live predicate (expr<=0) via a negated
            # iota, since only is_gt has codegen support
            nc.gpsimd.affine_select(
                out=mb_sb[:, h * QC:(h + 1) * QC],
                in_=mb_sb[:, h * QC:(h + 1) * QC],
                compare_op=is_gt,
                fill=-1.0e5,
                base=1 - P * (2 + h),
                pattern=[[1, QC]],
                channel_multiplier=-1,
            )
        # contiguous copy of the final trimmed pair's live-region mask so
        # the tail masking is a single tensor_tensor over [QC/2, QC+QC/4)
        mbt_sb = const.tile([P, 3 * QC // 4], f32)
        nc.vector.tensor_copy(mbt_sb[:, 0:QC // 2], mb_sb[:, QC // 2:QC])
        nc.vector.tensor_copy(
            mbt_sb[:, QC // 2:], mb_sb[:, 2 * QC - QC // 4:]
        )
        if not GMASK:
            dm8_sb = const.tile([P, 2 * QC], f8)
            nc.vector.tensor_copy(dm8_sb, dm_sb[:, 0, :])
        # tiny pass-through used to chain executions when benchmarking
        cht = const.tile([P, P], f32, name="cht")
        nc.scalar.dma_start(out=cht, in_=ch_h[:])
        nc.scalar.dma_start(out=cho_h[:], in_=cht)

        # ---- persistent activations ----
        qk_sb = pers.tile([P, S], f16)    # rows 0:64 Q^T, rows 64:128 K^T
        kt2_sb = pers.tile([64, S], f16)  # K^T relocated to partitions 0-63
        if ROWPACK_SCORES:
            qt2_sb = pers.tile([P, S], f16)  # rows 64:128 = Q^T duplicate
        v_sb = pers.tile([P, NT, VW], f16)  # V natural tiles + ones column
        nc.vector.memset(v_sb[:, :, 64:65], 1.0)
        # fp8 copy of V, DoubleRow-interleaved by tile pair (+ ones col):
        # steady-state (non-diagonal) PV runs as one fp8 DoubleRow matmul
        # per pair, halving its PE stream time.  Softmax averaging over
        # >=512 keys keeps the fp8 V/P noise ~0.2% on those pairs; the
        # few-key diagonal pairs stay on the fp16 path.
        v8_sb = pers.tile([P, NT // 2, 2, 80], f8)
        nc.vector.memset(v8_sb[:, :, :, 64:65], 1.0)

        xtc_tiles = {}
        po_tiles = {}

        def emit_xtc_dma(c):
            if c >= NQ:
                return
            t = xt_pool.tile([P, ND, QC], f16, name="xtc", tag="xtc")
            nc.sync.dma_start(out=t, in_=xt_h[:, c, :, :])
            xtc_tiles[c] = t

        def qk_store(c, ps_qk):
            """PSUM->SBUF move of packed Q^T/K^T on ScalarE (the DVE queue
            carries the latency-critical Schraudolph exp halves)."""
            cs = slice(c * QC, (c + 1) * QC)
            nc.scalar.copy(qk_sb[:, cs], ps_qk)
            # relocations (partition shifts need a DMA, not a DVE op); on
            # the sync ring so the gpsimd queue stays free for the
            # latency-critical diagonal masks
            nc.sync.dma_start(out=kt2_sb[:, cs], in_=qk_sb[64:128, cs])
            if ROWPACK_SCORES:
                nc.sync.dma_start(out=qt2_sb[64:128, cs], in_=qk_sb[0:64, cs])

        def proj0_gen():
            """Chunk-0 projection (fp16)."""
            xtc = xtc_tiles.pop(0)
            ps_qk = ps_sm.tile([P, QC], f32, name="ps_qk", tag="sm")
            for dc in range(ND):
                nc.tensor.matmul(
                    ps_qk, lhsT=wqk_sb[:, dc, :], rhs=xtc[:, dc, :],
                    start=(dc == 0), stop=(dc == ND - 1),
                )
                if dc % 2 == 1:
                    yield
            qk_store(0, ps_qk)
            yield
            # V projection: two col-tiled halves into one PSUM tile
            ps_v = ps_sm.tile([P, QC // 2], f32, name="ps_v", tag="sm")
            for dc in range(ND):
                st, sp = (dc == 0), (dc == ND - 1)
                nc.tensor.matmul(
                    ps_v[0:64, :], lhsT=wv_sb[:, dc, :],
                    rhs=xtc[:, dc, 0:QC // 2], start=st, stop=sp,
                )
                nc.tensor.matmul(
                    ps_v[64:128, :], lhsT=wv_sb[:, dc, :],
                    rhs=xtc[:, dc, QC // 2:], start=st, stop=sp,
                    tile_position=(0, 64),
                )
                if dc % 4 == 3:
                    yield
            vt_sb = stage.tile([P, QC // 2], f16, name="vt_sb", tag="vt")
            nc.vector.tensor_copy(vt_sb, ps_v)
            yield
            yield from v_epilog(0, vt_sb)
            emit_xtc_dma(2)

        def v_epilog(c, vt_sb):
            """Shared V-projection tail: transposes -> v_sb/v8_sb tiles.
            The [128,128] transposes run on the DMA xbar engine instead of
            the PE, freeing PE stream time; the copies then read SBUF
            (faster DVE perf modes than PSUM sources)."""
            # transpose [128,128] once per half: rows 0:64 of the result
            # are t-tile 4c+k, rows 64:128 are t-tile 4c+2+k
            for k in range(2):
                ps_tv = ps_sm.tile([P, P], f16, name="ps_tv", tag="sm")
                nc.tensor.transpose(ps_tv, vt_sb[:, k * P:(k + 1) * P], ident16)
                src2 = ps_tv[:, :].rearrange("p (a b) -> p a b", a=2)
                # merged strided copies: one op covers both t-tiles
                nc.vector.tensor_copy(
                    v_sb[:, 4 * c + k:4 * c + k + 3:2, 0:64], src2
                )
                nc.vector.tensor_copy(
                    v8_sb[:, 2 * c:2 * c + 2, k, 0:64], src2
                )
                yield

        def qkproj_gen(c):
            """Q,K projection of chunk c>=1 (packed, M=128)."""
            xtc = xtc_tiles[c]
            ps_qk = ps_sm.tile([P, QC], f32, name="ps_qk", tag="sm")
            for dc in range(ND):
                nc.tensor.matmul(
                    ps_qk, lhsT=wqk_sb[:, dc, :], rhs=xtc[:, dc, :],
                    start=(dc == 0), stop=(dc == ND - 1),
                )
                if dc % 2 == 1:
                    yield
            qk_store(c, ps_qk)
            yield

        def vproj_gen(c):
            """V projection of chunk c>=1: two col-tiled halves."""
            xtc = xtc_tiles.pop(c)
            ps_v = ps_sm.tile([P, QC // 2], f32, name="ps_v", tag="sm")
            for dc in range(ND):
                st, sp = (dc == 0), (dc == ND - 1)
                nc.tensor.matmul(
                    ps_v[0:64, :], lhsT=wv_sb[:, dc, :],
                    rhs=xtc[:, dc, 0:QC // 2], start=st, stop=sp,
                )
                nc.tensor.matmul(
                    ps_v[64:128, :], lhsT=wv_sb[:, dc, :],
                    rhs=xtc[:, dc, QC // 2:], start=st, stop=sp,
                    tile_position=(0, 64),
                )
                if dc % 4 == 3:
                    yield
            vt_sb = stage.tile([P, QC // 2], f16, name="vt_sb", tag="vt")
            nc.vector.tensor_copy(vt_sb, ps_v)
            yield
            yield from v_epilog(c, vt_sb)
            emit_xtc_dma(c + 2)

        def emit_pair(c, jp, po):
            """Emit the score matmuls + exp (+ masks) for pair jp of chunk
            c.  Returns a closure that emits the PV matmul(s); the caller
            defers it by two pairs so the PE queue always has ready score
            work ahead of a PV that stalls on its exp -- an in-order queue
            would otherwise expose the PE pipeline-fill latency on every
            exp wait."""
            cs = slice(c * QC, (c + 1) * QC)
            njs = 4 * (c + 1)
            j0, j1 = 2 * jp, 2 * jp + 1
            jj = j0 - 4 * c
            trimmed = jj == 2  # second diagonal pair: >62% masked
            # pair 0 avoids the row-packed path so a fresh chunk's first
            # scores don't wait on the qt2 relocation DMA
            packed = ROWPACK_SCORES and jp > 0

            def ktlo(j):
                return kt2_sb[:, j * P:(j + 1) * P]

            def qthi(lo):
                return qt2_sb[64:128, c * QC + lo:(c + 1) * QC]

            ps_s = ps_pair.tile([P, 2 * QC], f32, name="ps_s", tag="pair")
            if trimmed or (jj == 0 and c == 0):
                pt = pt_pool.tile([P, 2 * QC], f16, name="pt", tag="pt")
            if trimmed:
                # jj=2 half: only q in [256,512) is live; jj=3 half: only q in
                # [384,512), remapped to columns [512,640) so one ACT covers a
                # contiguous [256,640) region.
                nc.tensor.matmul(
                    ps_s[:, QC // 2:QC],
                    lhsT=ktlo(j0),
                    rhs=qk_sb[0:64, c * QC + QC // 2:(c + 1) * QC],
                    start=True, stop=True,
                )
                nc.tensor.matmul(
                    ps_s[:, QC:QC + QC // 4],
                    lhsT=qk_sb[64:128, j1 * P:(j1 + 1) * P],
                    rhs=qthi(3 * QC // 4),
                    start=True, stop=True, tile_position=(64, 0),
                )
                nc.gpsimd.memset(pt[:, 0:QC // 2], 0.0)
                if c == NQ - 1:
                    # final pair: mask pre-exp (drained pipeline; shortens
                    # the serial ACT->mask->PV tail chain)
                    nc.vector.tensor_tensor(
                        ps_s[:, QC // 2:QC + QC // 4],
                        ps_s[:, QC // 2:QC + QC // 4], mbt_sb, op=add_op,
                    )
                    nc.scalar.activation(
                        pt[:, QC // 2:QC + QC // 4],
                        ps_s[:, QC // 2:QC + QC // 4], Exp, scale=SCALE,
                    )
                else:
                    nc.scalar.activation(
                        pt[:, QC // 2:QC + QC // 4],
                        ps_s[:, QC // 2:QC + QC // 4], Exp, scale=SCALE,
                    )
                    # causal trimming on GpSimd (both slices reduce to the
                    # same local predicate: live where col >= partition)
                    for off, w in ((QC // 2, QC // 2), (QC, QC // 4)):
                        nc.gpsimd.affine_select(
                            out=pt[:, off:off + w], in_=pt[:, off:off + w],
                            compare_op=is_gt, fill=0.0,
                            base=1, pattern=[[1, w]],
                            channel_multiplier=-1,
                        )
                def pv_trimmed():
                    nc.tensor.matmul(
                        po[:, 3 * QC // 4:], lhsT=v_sb[:, j1, 0:65],
                        rhs=pt[:, QC:QC + QC // 4], start=False, stop=False,
                    )
                    nc.tensor.matmul(
                        po, lhsT=v_sb[:, j0, 0:65], rhs=pt[:, 0:QC],
                        start=False, stop=(j1 == njs - 1),
                    )
                return pv_trimmed
            nc.tensor.matmul(
                ps_s[:, 0:QC],
                lhsT=ktlo(j0), rhs=qk_sb[0:64, cs],
                start=True, stop=True,
            )
            if packed:
                nc.tensor.matmul(
                    ps_s[:, QC:],
                    lhsT=qk_sb[64:128, j1 * P:(j1 + 1) * P],
                    rhs=qthi(0),
                    start=True, stop=True, tile_position=(64, 0),
                )
            else:
                nc.tensor.matmul(
                    ps_s[:, QC:],
                    lhsT=ktlo(j1), rhs=qk_sb[0:64, cs],
                    start=True, stop=True,
                )
            if jj == 0 and c == 0:  # chunk-0 diagonal pair: fp16 + mask
                nc.scalar.activation(pt, ps_s, Exp, scale=SCALE)
                nc.vector.tensor_tensor(pt, pt, dm_sb[:, 0, :], op=mult)

                def pv_c0():
                    nc.tensor.matmul(
                        po, lhsT=v_sb[:, j0, 0:65], rhs=pt[:, 0:QC],
                        start=(j0 == 0), stop=False,
                    )
                    nc.tensor.matmul(
                        po, lhsT=v_sb[:, j1, 0:65], rhs=pt[:, QC:],
                        start=False, stop=(j1 == njs - 1),
                    )
                return pv_c0
            # fp8 exp tile, flat [P, 2QC]; the PV matmul reads it through a
            # DoubleRow-interleaved [P, 2, QC] view
            pt8 = pt8_pool.tile([P, 2 * QC], f8, name="pt8", tag="pt8")
            diag = jj == 0  # c >= 1 here (c == 0 handled above)
            if SCHRAB:
                # split exp: ScalarE ACTIVATE on the first THETA columns,
                # DVE Schraudolph exp2 bit trick on the rest.  Both run
                # concurrently, halving the per-pair exp latency and
                # splitting the elementwise load across the two engines.
                th = QC if diag else THETA
                nc.scalar.activation(
                    pt8[:, 0:th], ps_s[:, 0:th], Exp, scale=SCALE
                )
                nc.vector.tensor_scalar(
                    pt8[:, th:].bitcast(i8), ps_s[:, th:],
                    SCHRA_A, SCHRA_B, op0=mult, op1=add_op,
                )
            else:
                nc.scalar.activation(pt8, ps_s, Exp, scale=SCALE)
            if diag:
                if GMASK:
                    # in-place causal zeroing on GpSimd: keeps where
                    # query >= key, fills 0 above the diagonal
                    for ko in range(2):
                        nc.gpsimd.affine_select(
                            out=pt8[:, ko * QC:(ko + 1) * QC],
                            in_=pt8[:, ko * QC:(ko + 1) * QC],
                            compare_op=is_gt, fill=0.0,
                            base=1 - P * ko,
                            pattern=[[1, QC]],
                            channel_multiplier=-1,
                        )
                else:
                    nc.vector.tensor_tensor(pt8, pt8, dm8_sb, op=mult)

            def pv_dr():
                nc.tensor.matmul(
                    po, lhsT=v8_sb[:, jp, :, 0:65],
                    rhs=pt8[:, :].rearrange("p (a b) -> p a b", a=2),
                    start=(j0 == 0), stop=(j1 == njs - 1),
                    perf_mode=DR,
                )
            return pv_dr

        def epi_gen(c):
            """Store U^T + denominators for chunk c; the host normalizes."""
            last = c == NQ - 1
            po = po_tiles.pop(c)
            osb = stage.tile([65, QC], f32, name="osb", tag="osb")
            nc.scalar.copy(osb, po)
            yield
            eng = nc.scalar if last else nc.sync
            eng.dma_start(out=o_h[c], in_=osb)
            yield

        # Software pipeline: a global queue of deferrable PE work
        # (projections of later chunks, epilogues of finished chunks) is
        # drained in small bursts between attention pairs, so the PE fills
        # its exp-wait slack and never idles across chunk boundaries.
        proj_pending = []   # [(due, generator)] sorted by deadline
        epi_pending = []    # generators (no deadline)

        def pull_one(max_due=None):
            while proj_pending:
                due, g = proj_pending[0]
                if max_due is not None and due > max_due:
                    break
                try:
                    next(g)
                    return
                except StopIteration:
                    proj_pending.pop(0)
            while epi_pending:
                try:
                    next(epi_pending[0])
                    return
                except StopIteration:
                    epi_pending.pop(0)

        def ensure(due):
            while proj_pending and proj_pending[0][0] <= due:
                _, g = proj_pending[0]
                for _ in g:
                    pass
                proj_pending.pop(0)

        xtc_tiles[0] = xtc0
        emit_xtc_dma(1)
        for _ in proj0_gen():
            pass
        # deadline-ordered deferred PE work: qkproj(c) is due at chunk-c
        # start, vproj(c) only at chunk c's first diagonal pair (pair 2c),
        # which spreads projection bursts across the ACT-paced pair loop
        for c in range(1, NQ):
            proj_pending.append(((c, 0), qkproj_gen(c)))
            proj_pending.append(((c, 1), vproj_gen(c)))
        proj_pending.sort(key=lambda t: t[0])
        for c in range(NQ):
            ensure((c, 0))
            po = ps_op.tile([65, QC], f32, name="po", tag="po")
            po_tiles[c] = po
            pv_queue = []
            for jp in range(2 * (c + 1)):
                if jp == 2 * c:
                    ensure((c, 1))
                pv_queue.append(emit_pair(c, jp, po))
                if len(pv_queue) > 2:
                    pv_queue.pop(0)()
                # front-load upcoming projections into the pair slack so
                # they (and the relocation DMAs) beat their deadlines
                pulls = 3 if jp < 4 else 1
                for _ in range(pulls):
                    pull_one(max_due=(c + 1, 0))
            while pv_queue:
                pv_queue.pop(0)()
            epi_pending.append(epi_gen(c))
        for _, g in proj_pending:
            for _ in g:
                pass
        proj_pending.clear()
        for g in epi_pending:
            for _ in g:
                pass
    nc.finalize()
    return nc


def _dev_w(w):
    # [D, m] -> [P, ND, m] with d = dc*P + p
    w = np.ascontiguousarray(w).astype(np.float16)
    return np.ascontiguousarray(w.reshape(ND, P, -1).transpose(1, 0, 2))


def _host_inputs(x, wQ, wK, wV):
    x = np.asarray(x, dtype=np.float32)
    wqk = _dev_w(np.concatenate([np.asarray(wQ).T, np.asarray(wK).T], axis=1))
    wv = _dev_w(np.asarray(wV).T)
    ww = np.ascontiguousarray(np.concatenate([wqk, wv], axis=2))
    chain = np.zeros((P, P), np.float32)
    in_maps = []
    for b in range(B):
        # x_b.T [D, S] -> [P, NQ, ND, QC] with d = dc*P + p, s = c*QC + sc
        xt = np.ascontiguousarray(
            x[b].T.astype(np.float16)
            .reshape(ND, P, NQ, QC)
            .transpose(1, 2, 0, 3)
        )
        in_maps.append({"xt": xt, "ww": ww, "chain": chain})
    return in_maps


def kernel(x, wQ, wK, wV):
    from concourse.bass_utils import run_bass_kernel_spmd

    if "nc" not in _CACHED:
        _CACHED["nc"] = _build_nc()
    nc = _CACHED["nc"]
    in_maps = _host_inputs(x, wQ, wK, wV)
    res = run_bass_kernel_spmd(nc, in_maps, core_ids=list(range(B)))
    out = np.empty((B, S, DK), np.float32)
    for b in range(B):
        u = res.results[b]["o"]          # [NQ, 65, QC]
        o = u[:, :64, :] / u[:, 64:65, :]
        out[b] = o.transpose(0, 2, 1).reshape(S, DK)
    return out


# revision 33
# speedup vs baseline: 1.1594x; 1.0061x over previous
"""Trainium2 Bass kernel for a single causal attention head.

Reference computation (per batch element b):
    Q = x_b @ wQ.T ; K = x_b @ wK.T ; V = x_b @ wV.T          [S, DK]
    P = softmax(causal_mask(Q @ K.T * d_model**-0.5))          [S, S]
    O = P @ V                                                  [S, DK]

Sharding: one batch element per NeuronCore (B == n_cores == 8).
Weights are replicated. No collectives needed.

Per-core device layout (host pre-transposes/casts for PE-friendly fp16):
    xt    [D, S]   fp16  x_b transposed (contraction dim D on partitions)
    wqk   [D, 128] fp16  [wQ.T | wK.T]  -> packed projection, M=128
    wv    [D, 64]  fp16  wV.T
Output:
    o     [NQ, 65, QC] fp32  unnormalized U^T rows 0:64 + softmax
                             denominators in row 64; the host divides and
                             transposes during the gather/unshard step
                             (flash-attention partial-merge convention).

Device pipeline per 512-wide q-chunk c (fp16 matmuls, fp32 PSUM):
  - DMA xt chunk; project Q^T,K^T packed (M=128); project V^T with
    col-tiled half-chunks into one PSUM tile.
  - Duplicate Q^T/K^T across both partition halves (SBUF->SBUF DMA) so
    score matmuls for t-tiles j,j+1 run row-packed (rows 0-63 / 64-127
    of the PE array concurrently), writing one [128,1024] PSUM pair.
  - exp: the ScalarE ACTIVATE is the kernel bottleneck, so the exp work
    is split across two engines: even steady pairs use the ScalarE exp
    ACTIVATE -> fp8; odd steady pairs compute exp on the Vector engine
    with the Schraudolph exp2 bit trick (int8(z*8/ln2 + bias) bitcast
    to fp8e4m3 ~= 2^z with ~3% multiplicative ripple that washes out in
    the softmax average).
  - Diagonal pairs: ScalarE exp -> fp8, causal-masked in place by
    GpSimd affine_selects (keeps the Vector engine free).
  - PV: accumulate [V_j | 1].T @ P~_j into PSUM (U^T rows 0-63,
    softmax denominators in row 64) with fp8 DoubleRow matmuls.
"""

import numpy as np

B, S, D, DK = 8, 4096, 1024, 64
P = 128
QC = 512          # q-chunk width (matmul moving dim)
NQ = S // QC      # 8 q-chunks
ND = D // P       # 8 contraction chunks
NT = S // P       # 32 t-tiles
SCALE = float(D) ** -0.5   # 1/32
VW = 66           # per-t-tile stride in v_sb (64 V cols + 1 ones + pad)

# Schraudolph exp2-bit-trick constants for the DVE exp path:
# i8 = trunc(score * SCHRA_A + SCHRA_B); bitcast(i8) as fp8e4m3 ~ exp(score/32)
SCHRA_A = SCALE * 8.0 * 1.4426950408889634
SCHRA_B = 55.95

# optimization knobs (validated on HW; flip off if a variant fails)
ROWPACK_SCORES = True   # row-packed score matmul pairs
SCHRAB = True           # split each pair's exp: ScalarE half + DVE half
GMASK = True            # diagonal-pair causal masks on GpSimd
THETA = 512             # exp split point: ScalarE gets [0:THETA) of 1024

_CACHED = {}


def _build_nc():
    import concourse.mybir as mybir
    import concourse.tile as tile
    from concourse import bacc
    from concourse.masks import make_identity
    from contextlib import ExitStack

    f32 = mybir.dt.float32
    f16 = mybir.dt.float16
    f8 = mybir.dt.float8e4
    i8 = mybir.dt.int8
    DR = mybir.MatmulPerfMode.DoubleRow
    Exp = mybir.ActivationFunctionType.Exp
    add_op = mybir.AluOpType.add
    mult = mybir.AluOpType.mult
    is_gt = mybir.AluOpType.is_gt

    nc = bacc.Bacc()
    # host pre-layouts: per-partition contiguous so DMAs need no gather.
    # ww packs [wqk | wv] along the last dim -> one DMA issue.
    WW = P + DK
    xt_h = nc.declare_dram_parameter("xt", [P, NQ, ND, QC], f16, isOutput=False)
    ww_h = nc.declare_dram_parameter("ww", [P, ND, WW], f16, isOutput=False)
    ch_h = nc.declare_dram_parameter("chain", [P, P], f32, isOutput=False)
    o_h = nc.declare_dram_parameter("o", [NQ, 65, QC], f32, isOutput=True)
    cho_h = nc.declare_dram_parameter("chain_o", [P, P], f32, isOutput=True)

    with tile.TileContext(nc) as tc, ExitStack() as ctx:
        const = ctx.enter_context(tc.tile_pool(name="const", bufs=1))
        xt_pool = ctx.enter_context(tc.tile_pool(name="xtp", bufs=2))
        pers = ctx.enter_context(tc.tile_pool(name="pers", bufs=1))
        pt_pool = ctx.enter_context(tc.tile_pool(name="ptp", bufs=3))
        pt8_pool = ctx.enter_context(tc.tile_pool(name="ptp8", bufs=6))
        stage = ctx.enter_context(tc.tile_pool(name="stage", bufs=2))
        # PSUM budget (8 banks): pair pool 3x2 + po 1 + sm 1 = 8.  Three
        # score buffers let the PE run up to three pairs ahead of the exp
        # engines, hiding per-matmul pipeline latency and exp jitter.
        ps_pair = ctx.enter_context(tc.tile_pool(name="ps_pair", bufs=3, space="PSUM"))
        ps_op = ctx.enter_context(tc.tile_pool(name="ps_op", bufs=1, space="PSUM"))
        ps_sm = ctx.enter_context(tc.tile_pool(name="ps_sm", bufs=1, space="PSUM"))

        # ---- weights first (small, gate the first projection), then the
        # first x chunk in quarters so the projection starts on the first
        # quarter while the rest streams in ----
        ww_sb = const.tile([P, ND, WW], f16)
        nc.sync.dma_start(out=ww_sb, in_=ww_h[:])
        wqk_sb = ww_sb[:, :, 0:P]
        wv_sb = ww_sb[:, :, P:]
        xtc0 = xt_pool.tile([P, ND, QC], f16, name="xtc", tag="xtc")
        for h in range(4):
            nc.sync.dma_start(
                out=xtc0[:, 2 * h:2 * (h + 1), :],
                in_=xt_h[:, 0, 2 * h:2 * (h + 1), :],
            )
        # ---- PE warm-up: dummy matmuls during the initial DMA wait so the
        # HAM clock gate is already at full rate (2.4 GHz) when the first
        # projection runs.  Depends only on one vector memset. ----
        warm_sb = const.tile([P, QC], f16, name="warm_sb")
        nc.vector.memset(warm_sb, 0.0)
        ps_warm = ps_sm.tile([P, QC], f32, name="ps_warm", tag="sm")
        for _ in range(3):
            nc.tensor.matmul(ps_warm, lhsT=warm_sb[:, 0:P], rhs=warm_sb,
                             start=True, stop=True)
        # identity before the causal masks: ident16 gates the first V
        # transpose (PE critical path)
        ident16 = const.tile([P, P], f16)
        make_identity(nc, ident16)
        # causal masks: fp16 multiplicative (post-exp) for the fp16-path
        # pairs, plus an fp32 additive bias (pre-exp) used only on the final
        # trimmed pair where it shortens the drained tail chain
        dm_sb = const.tile([P, 2, 2 * QC], f16)
        nc.gpsimd.memset(dm_sb, 0.0)
        for g in range(2):
            for h in range(2):
                nc.gpsimd.affine_select(
                    out=dm_sb[:, g, h * QC:(h + 1) * QC],
                    in_=dm_sb[:, g, h * QC:(h + 1) * QC],
                    compare_op=is_gt,
                    fill=1.0,
                    base=P * (2 * g + h),
                    pattern=[[-1, QC]],
                    channel_multiplier=1,
                )
        mb_sb = const.tile([P, 2 * QC], f32)
        nc.gpsimd.memset(mb_sb, 0.0)
        for h in range(2):
            # complement of the live predicate (expr<=0) via a negated
            # iota, since only is_gt has codegen support
            nc.gpsimd.affine_select(
                out=mb_sb[:, h * QC:(h + 1) * QC],
                in_=mb_sb[:, h * QC:(h + 1) * QC],
                compare_op=is_gt,
                fill=-1.0e5,
                base=1 - P * (2 + h),
                pattern=[[1, QC]],
                channel_multiplier=-1,
            )
        # contiguous copy of the final trimmed pair's live-region mask so
        # the tail masking is a single tensor_tensor over [QC/2, QC+QC/4)
        mbt_sb = const.tile([P, 3 * QC // 4], f32)
        nc.vector.tensor_copy(mbt_sb[:, 0:QC // 2], mb_sb[:, QC // 2:QC])
        nc.vector.tensor_copy(
            mbt_sb[:, QC // 2:], mb_sb[:, 2 * QC - QC // 4:]
        )
        if not GMASK:
            dm8_sb = const.tile([P, 2 * QC], f8)
            nc.vector.tensor_copy(dm8_sb, dm_sb[:, 0, :])
        # tiny pass-through used to chain executions when benchmarking
        cht = const.tile([P, P], f32, name="cht")
        nc.scalar.dma_start(out=cht, in_=ch_h[:])
        nc.scalar.dma_start(out=cho_h[:], in_=cht)

        # ---- persistent activations ----
        qk_sb = pers.tile([P, S], f16)    # rows 0:64 Q^T, rows 64:128 K^T
        kt2_sb = pers.tile([64, S], f16)  # K^T relocated to partitions 0-63
        if ROWPACK_SCORES:
            qt2_sb = pers.tile([P, S], f16)  # rows 64:128 = Q^T duplicate
        v_sb = pers.tile([P, NT, VW], f16)  # V natural tiles + ones column
        nc.vector.memset(v_sb[:, :, 64:65], 1.0)
        # fp8 copy of V, DoubleRow-interleaved by tile pair (+ ones col):
        # steady-state (non-diagonal) PV runs as one fp8 DoubleRow matmul
        # per pair, halving its PE stream time.  Softmax averaging over
        # >=512 keys keeps the fp8 V/P noise ~0.2% on those pairs; the
        # few-key diagonal pairs stay on the fp16 path.
        v8_sb = pers.tile([P, NT // 2, 2, 80], f8)
        nc.vector.memset(v8_sb[:, :, :, 64:65], 1.0)

        xtc_tiles = {}
        po_tiles = {}

        def emit_xtc_dma(c):
            if c >= NQ:
                return
            t = xt_pool.tile([P, ND, QC], f16, name="xtc", tag="xtc")
            nc.sync.dma_start(out=t, in_=xt_h[:, c, :, :])
            xtc_tiles[c] = t

        def qk_store(c, ps_qk):
            """PSUM->SBUF move of packed Q^T/K^T on ScalarE (the DVE queue
            carries the latency-critical Schraudolph exp halves)."""
            cs = slice(c * QC, (c + 1) * QC)
            nc.scalar.copy(qk_sb[:, cs], ps_qk)
            # relocations (partition shifts need a DMA, not a DVE op); on
            # the sync ring so the gpsimd queue stays free for the
            # latency-critical diagonal masks
            nc.sync.dma_start(out=kt2_sb[:, cs], in_=qk_sb[64:128, cs])
            if ROWPACK_SCORES:
                nc.sync.dma_start(out=qt2_sb[64:128, cs], in_=qk_sb[0:64, cs])

        def proj0_gen():
            """Chunk-0 projection (fp16)."""
            xtc = xtc_tiles.pop(0)
            ps_qk = ps_sm.tile([P, QC], f32, name="ps_qk", tag="sm")
            for dc in range(ND):
                nc.tensor.matmul(
                    ps_qk, lhsT=wqk_sb[:, dc, :], rhs=xtc[:, dc, :],
                    start=(dc == 0), stop=(dc == ND - 1),
                )
                if dc % 2 == 1:
                    yield
            qk_store(0, ps_qk)
            yield
            # V projection: two col-tiled halves into one PSUM tile
            ps_v = ps_sm.tile([P, QC // 2], f32, name="ps_v", tag="sm")
            for dc in range(ND):
                st, sp = (dc == 0), (dc == ND - 1)
                nc.tensor.matmul(
                    ps_v[0:64, :], lhsT=wv_sb[:, dc, :],
                    rhs=xtc[:, dc, 0:QC // 2], start=st, stop=sp,
                )
                nc.tensor.matmul(
                    ps_v[64:128, :], lhsT=wv_sb[:, dc, :],
                    rhs=xtc[:, dc, QC // 2:], start=st, stop=sp,
                    tile_position=(0, 64),
                )
                if dc % 4 == 3:
                    yield
            vt_sb = stage.tile([P, QC // 2], f16, name="vt_sb", tag="vt")
            nc.vector.tensor_copy(vt_sb, ps_v)
            yield
            yield from v_epilog(0, vt_sb)
            emit_xtc_dma(2)

        def v_epilog(c, vt_sb):
            """Shared V-projection tail: transposes -> v_sb/v8_sb tiles.
            The [128,128] transposes run on the DMA xbar engine instead of
            the PE, freeing PE stream time; the copies then read SBUF
            (faster DVE perf modes than PSUM sources)."""
            # transpose [128,128] once per half: rows 0:64 of the result
            # are t-tile 4c+k, rows 64:128 are t-tile 4c+2+k
            for k in range(2):
                ps_tv = ps_sm.tile([P, P], f16, name="ps_tv", tag="sm")
                nc.tensor.transpose(ps_tv, vt_sb[:, k * P:(k + 1) * P], ident16)
                src2 = ps_tv[:, :].rearrange("p (a b) -> p a b", a=2)
                # merged strided copies: one op covers both t-tiles
                nc.vector.tensor_copy(
                    v_sb[:, 4 * c + k:4 * c + k + 3:2, 0:64], src2
                )
                nc.vector.tensor_copy(
                    v8_sb[:, 2 * c:2 * c + 2, k, 0:64], src2
                )
                yield

        def qkproj_gen(c):
            """Q,K projection of chunk c>=1 (packed, M=128)."""
            xtc = xtc_tiles[c]
            ps_qk = ps_sm.tile([P, QC], f32, name="ps_qk", tag="sm")
            for dc in range(ND):
                nc.tensor.matmul(
                    ps_qk, lhsT=wqk_sb[:, dc, :], rhs=xtc[:, dc, :],
                    start=(dc == 0), stop=(dc == ND - 1),
                )
                if dc % 2 == 1:
                    yield
            qk_store(c, ps_qk)
            yield

        def vproj_gen(c):
            """V projection of chunk c>=1: two col-tiled halves."""
            xtc = xtc_tiles.pop(c)
            ps_v = ps_sm.tile([P, QC // 2], f32, name="ps_v", tag="sm")
            for dc in range(ND):
                st, sp = (dc == 0), (dc == ND - 1)
                nc.tensor.matmul(
                    ps_v[0:64, :], lhsT=wv_sb[:, dc, :],
                    rhs=xtc[:, dc, 0:QC // 2], start=st, stop=sp,
                )
                nc.tensor.matmul(
                    ps_v[64:128, :], lhsT=wv_sb[:, dc, :],
                    rhs=xtc[:, dc, QC // 2:], start=st, stop=sp,
                    tile_position=(0, 64),
                )
                if dc % 4 == 3:
                    yield
            vt_sb = stage.tile([P, QC // 2], f16, name="vt_sb", tag="vt")
            nc.vector.tensor_copy(vt_sb, ps_v)
            yield
            yield from v_epilog(c, vt_sb)
            emit_xtc_dma(c + 2)

        def emit_pair(c, jp, po):
            """Emit the score matmuls + exp (+ masks) for pair jp of chunk
            c.  Returns a closure that emits the PV matmul(s); the caller
            defers it by two pairs so the PE queue always has ready score
            work ahead of a PV that stalls on its exp -- an in-order queue
            would otherwise expose the PE pipeline-fill latency on every
            exp wait."""
            cs = slice(c * QC, (c + 1) * QC)
            njs = 4 * (c + 1)
            j0, j1 = 2 * jp, 2 * jp + 1
            jj = j0 - 4 * c
            trimmed = jj == 2  # second diagonal pair: >62% masked
            # pair 0 avoids the row-packed path so a fresh chunk's first
            # scores don't wait on the qt2 relocation DMA
            packed = ROWPACK_SCORES and jp > 0

            def ktlo(j):
                return kt2_sb[:, j * P:(j + 1) * P]

            def qthi(lo):
                return qt2_sb[64:128, c * QC + lo:(c + 1) * QC]

            ps_s = ps_pair.tile([P, 2 * QC], f32, name="ps_s", tag="pair")
            if trimmed or (jj == 0 and c == 0):
                pt = pt_pool.tile([P, 2 * QC], f16, name="pt", tag="pt")
            if trimmed:
                # jj=2 half: only q in [256,512) is live; jj=3 half: only q in
                # [384,512), remapped to columns [512,640) so one ACT covers a
                # contiguous [256,640) region.
                nc.tensor.matmul(
                    ps_s[:, QC // 2:QC],
                    lhsT=ktlo(j0),
                    rhs=qk_sb[0:64, c * QC + QC // 2:(c + 1) * QC],
                    start=True, stop=True,
                )
                nc.tensor.matmul(
                    ps_s[:, QC:QC + QC // 4],
                    lhsT=qk_sb[64:128, j1 * P:(j1 + 1) * P],
                    rhs=qthi(3 * QC // 4),
                    start=True, stop=True, tile_position=(64, 0),
                )
                nc.gpsimd.memset(pt[:, 0:QC // 2], 0.0)
                if c == NQ - 1:
                    # final pair: mask pre-exp (drained pipeline; shortens
                    # the serial ACT->mask->PV tail chain)
                    nc.vector.tensor_tensor(
                        ps_s[:, QC // 2:QC + QC // 4],
                        ps_s[:, QC // 2:QC + QC // 4], mbt_sb, op=add_op,
                    )
                    nc.scalar.activation(
                        pt[:, QC // 2:QC + QC // 4],
                        ps_s[:, QC // 2:QC + QC // 4], Exp, scale=SCALE,
                    )
                else:
                    nc.scalar.activation(
                        pt[:, QC // 2:QC + QC // 4],
                        ps_s[:, QC // 2:QC + QC // 4], Exp, scale=SCALE,
                    )
                    # causal trimming on GpSimd (both slices reduce to the
                    # same local predicate: live where col >= partition)
                    for off, w in ((QC // 2, QC // 2), (QC, QC // 4)):
                        nc.gpsimd.affine_select(
                            out=pt[:, off:off + w], in_=pt[:, off:off + w],
                            compare_op=is_gt, fill=0.0,
                            base=1, pattern=[[1, w]],
                            channel_multiplier=-1,
                        )
                def pv_trimmed():
                    nc.tensor.matmul(
                        po[:, 3 * QC // 4:], lhsT=v_sb[:, j1, 0:65],
                        rhs=pt[:, QC:QC + QC // 4], start=False, stop=False,
                    )
                    nc.tensor.matmul(
                        po, lhsT=v_sb[:, j0, 0:65], rhs=pt[:, 0:QC],
                        start=False, stop=(j1 == njs - 1),
                    )
                return pv_trimmed
            nc.tensor.matmul(
                ps_s[:, 0:QC],
                lhsT=ktlo(j0), rhs=qk_sb[0:64, cs],
                start=True, stop=True,
            )
            if packed:
                nc.tensor.matmul(
                    ps_s[:, QC:],
                    lhsT=qk_sb[64:128, j1 * P:(j1 + 1) * P],
                    rhs=qthi(0),
                    start=True, stop=True, tile_position=(64, 0),
                )
            else:
                nc.tensor.matmul(
                    ps_s[:, QC:],
                    lhsT=ktlo(j1), rhs=qk_sb[0:64, cs],
                    start=True, stop=True,
                )
            if jj == 0 and c == 0:  # chunk-0 diagonal pair: fp16 + mask
                nc.scalar.activation(pt, ps_s, Exp, scale=SCALE)
                nc.vector.tensor_tensor(pt, pt, dm_sb[:, 0, :], op=mult)

                def pv_c0():
                    nc.tensor.matmul(
                        po, lhsT=v_sb[:, j0, 0:65], rhs=pt[:, 0:QC],
                        start=(j0 == 0), stop=False,
                    )
                    nc.tensor.matmul(
                        po, lhsT=v_sb[:, j1, 0:65], rhs=pt[:, QC:],
                        start=False, stop=(j1 == njs - 1),
                    )
                return pv_c0
            # fp8 exp tile, flat [P, 2QC]; the PV matmul reads it through a
            # DoubleRow-interleaved [P, 2, QC] view
            pt8 = pt8_pool.tile([P, 2 * QC], f8, name="pt8", tag="pt8")
            diag = jj == 0  # c >= 1 here (c == 0 handled above)
            if SCHRAB:
                # split exp: ScalarE ACTIVATE on the first THETA columns,
                # DVE Schraudolph exp2 bit trick on the rest.  Both run
                # concurrently, halving the per-pair exp latency and
                # splitting the elementwise load across the two engines.
                th = QC if diag else THETA
                nc.scalar.activation(
                    pt8[:, 0:th], ps_s[:, 0:th], Exp, scale=SCALE
                )
                nc.vector.tensor_scalar(
                    pt8[:, th:].bitcast(i8), ps_s[:, th:],
                    SCHRA_A, SCHRA_B, op0=mult, op1=add_op,
                )
            else:
                nc.scalar.activation(pt8, ps_s, Exp, scale=SCALE)
            if diag:
                if GMASK:
                    # in-place causal zeroing on GpSimd: keeps where
                    # query >= key, fills 0 above the diagonal
                    for ko in range(2):
                        nc.gpsimd.affine_select(
                            out=pt8[:, ko * QC:(ko + 1) * QC],
                            in_=pt8[:, ko * QC:(ko + 1) * QC],
                            compare_op=is_gt, fill=0.0,
                            base=1 - P * ko,
                            pattern=[[1, QC]],
                            channel_multiplier=-1,
                        )
                else:
                    nc.vector.tensor_tensor(pt8, pt8, dm8_sb, op=mult)

            def pv_dr():
                nc.tensor.matmul(
                    po, lhsT=v8_sb[:, jp, :, 0:65],
                    rhs=pt8[:, :].rearrange("p (a b) -> p a b", a=2),
                    start=(j0 == 0), stop=(j1 == njs - 1),
                    perf_mode=DR,
                )
            return pv_dr

        def epi_gen(c):
            """Store U^T + denominators for chunk c; the host normalizes."""
            last = c == NQ - 1
            po = po_tiles.pop(c)
            osb = stage.tile([65, QC], f32, name="osb", tag="osb")
            nc.scalar.copy(osb, po)
            yield
            eng = nc.scalar if last else nc.sync
            eng.dma_start(out=o_h[c], in_=osb)
            yield

        # Software pipeline: a global queue of deferrable PE work
        # (projections of later chunks, epilogues of finished chunks) is
        # drained in small bursts between attention pairs, so the PE fills
        # its exp-wait slack and never idles across chunk boundaries.
        proj_pending = []   # [(due, generator)] sorted by deadline
        epi_pending = []    # generators (no deadline)

        def pull_one(max_due=None):
            while proj_pending:
                due, g = proj_pending[0]
                if max_due is not None and due > max_due:
                    break
                try:
                    next(g)
                    return
                except StopIteration:
                    proj_pending.pop(0)
            while epi_pending:
                try:
                    next(epi_pending[0])
                    return
                except StopIteration:
                    epi_pending.pop(0)

        def ensure(due):
            while proj_pending and proj_pending[0][0] <= due:
                _, g = proj_pending[0]
                for _ in g:
                    pass
                proj_pending.pop(0)

        xtc_tiles[0] = xtc0
        emit_xtc_dma(1)
        for _ in proj0_gen():
            pass
        # deadline-ordered deferred PE work: qkproj(c) is due at chunk-c
        # start, vproj(c) only at chunk c's first diagonal pair (pair 2c),
        # which spreads projection bursts across the ACT-paced pair loop
        for c in range(1, NQ):
            proj_pending.append(((c, 0), qkproj_gen(c)))
            proj_pending.append(((c, 1), vproj_gen(c)))
        proj_pending.sort(key=lambda t: t[0])
        # global two-deep PV pipeline: PVs trail their pair's scores/exp by
        # two pairs even across chunk boundaries, so the PE never drains at
        # a boundary.  An epilogue is registered only once its chunk's last
        # PV has been emitted (emitting the PSUM->SBUF copy earlier would
        # head-of-line-block the Scalar queue on that PV).
        pv_queue = []  # [(chunk, is_last_of_chunk, closure)]

        def pv_pop():
            cc, last_of, pv = pv_queue.pop(0)
            pv()
            if last_of:
                epi_pending.append(epi_gen(cc))

        for c in range(NQ):
            ensure((c, 0))
            po = ps_op.tile([65, QC], f32, name="po", tag="po")
            po_tiles[c] = po
            npair = 2 * (c + 1)
            for jp in range(npair):
                if jp == 2 * c:
                    ensure((c, 1))
                pv_queue.append((c, jp == npair - 1, emit_pair(c, jp, po)))
                if len(pv_queue) > 2:
                    pv_pop()
                # front-load upcoming projections into the pair slack so
                # they (and the relocation DMAs) beat their deadlines
                pulls = 3 if jp < 4 else 1
                for _ in range(pulls):
                    pull_one(max_due=(c + 1, 0))
        while pv_queue:
            pv_pop()
        for _, g in proj_pending:
            for _ in g:
                pass
        proj_pending.clear()
        for g in epi_pending:
            for _ in g:
                pass
    nc.finalize()
    return nc


def _dev_w(w):
    # [D, m] -> [P, ND, m] with d = dc*P + p
    w = np.ascontiguousarray(w).astype(np.float16)
    return np.ascontiguousarray(w.reshape(ND, P, -1).transpose(1, 0, 2))


def _host_inputs(x, wQ, wK, wV):
    x = np.asarray(x, dtype=np.float32)
    wqk = _dev_w(np.concatenate([np.asarray(wQ).T, np.asarray(wK).T], axis=1))
    wv = _dev_w(np.asarray(wV).T)
    ww = np.ascontiguousarray(np.concatenate([wqk, wv], axis=2))
    chain = np.zeros((P, P), np.float32)
    in_maps = []
    for b in range(B):
        # x_b.T [D, S] -> [P, NQ, ND, QC] with d = dc*P + p, s = c*QC + sc
        xt = np.ascontiguousarray(
            x[b].T.astype(np.float16)
            .reshape(ND, P, NQ, QC)
            .transpose(1, 2, 0, 3)
        )
        in_maps.append({"xt": xt, "ww": ww, "chain": chain})
    return in_maps


def kernel(x, wQ, wK, wV):
    from concourse.bass_utils import run_bass_kernel_spmd

    if "nc" not in _CACHED:
        _CACHED["nc"] = _build_nc()
    nc = _CACHED["nc"]
    in_maps = _host_inputs(x, wQ, wK, wV)
    res = run_bass_kernel_spmd(nc, in_maps, core_ids=list(range(B)))
    out = np.empty((B, S, DK), np.float32)
    for b in range(B):
        u = res.results[b]["o"]          # [NQ, 65, QC]
        o = u[:, :64, :] / u[:, 64:65, :]
        out[b] = o.transpose(0, 2, 1).reshape(S, DK)
    return out
